# revision 1
# baseline (speedup 1.0000x reference)
"""3-layer GAT + global-mean-pool + FC on 8 Trainium2 NeuronCores.

Strategy (graph/data parallel, per the sharding hint):
  - Nodes (and their incident edges, by dst) are sharded across 8 cores.
  - Per layer, each core computes h = x @ W_ext for its node slice (W_ext also
    produces the per-node attention terms al_src/al_dst as extra columns),
    writes the rows to a DRAM table, and an AllGather replicates the full
    table to every core.
  - Edges are dst-sorted into 128-dst windows; messages h[src] are fetched
    with dma_gather (row gather), attention logits use the gathered al_src
    plus a second small gather of the local al_dst table, and the per-window
    segment-sum (and softmax denominator) is one matmul per 128-edge block
    with a compare-built 0/1 edge->dst matrix as the stationary operand.
  - LayerNorm+ELU run batched on the node-major result; the final mean-pool
    is another compare-matmul followed by an AllReduce and a tiny FC.

Weight folding (al = h @ a  ->  x @ (W @ a)), channel interleaving and all
integer edge-structure preprocessing happen on the host; all O(N), O(E)
floating-point work runs on device.
"""

import os
import sys

GAT_LAYERS = int(os.environ.get("GAT_LAYERS", "3"))
GAT_NO_AG = os.environ.get("GAT_NO_AG", "0") == "1"
GAT_NO_GATHER = os.environ.get("GAT_NO_GATHER", "0") == "1"
GAT_NO_MM = os.environ.get("GAT_NO_MM", "0") == "1"
GAT_NO_AR = os.environ.get("GAT_NO_AR", "0") == "1"
GAT_NO_POOL = os.environ.get("GAT_NO_POOL", "0") == "1"
GAT_NO_INV = os.environ.get("GAT_NO_INV", "0") == "1"
GAT_NO_FC = os.environ.get("GAT_NO_FC", "0") == "1"
GAT_DEBUG = os.environ.get("GAT_DEBUG", "0") == "1"


for _p in ("/opt/trn_rl_repo", "/opt/pypackages"):
    if _p not in sys.path:
        sys.path.append(_p)

import numpy as np

import concourse.bass as bass
import concourse.bacc as bacc
import concourse.tile as tile
import concourse.mybir as mybir
from concourse import library_config
from concourse.bass_utils import run_bass_kernel_spmd

# ---- problem constants (hardcoded per spec) ----
N = 50000
E0 = 800000
NCORES = 8
PARTN = N // NCORES          # 6250 real nodes per core
EMBED = 256
HIDDEN = 64
HEADS = 4
NG = 64                      # graphs
EPS = 1e-5
NEG = 0.2
DW = 128                     # dst window
NW = (PARTN + DW - 1) // DW  # 49 windows per core
NT = NW                      # node tiles per core (node = 128*t + p)
PADN = NT * 128              # 6272 padded nodes per core (table rows per rank)
NPAD = NCORES * PADN         # 50176 global padded table rows
SPLIT = 32768                # int16 index split for src gather

F32 = mybir.dt.float32
BF16 = mybir.dt.bfloat16
I16 = mybir.dt.int16

# table dtype config
USE_BF16 = os.environ.get("GAT_BF16", "1") == "1"
if USE_BF16:
    TROW = 384               # bf16 cols per table row (768B, %256)
    TDT = BF16
    CHUNK = int(os.environ.get("GAT_CHUNK", "8"))
else:
    TROW = 320               # f32 cols per table row (1280B, %256)
    TDT = F32
    CHUNK = int(os.environ.get("GAT_CHUNK", "6"))

ALS0 = 256                   # col where al_src lives in the table row (f32)
# consts tile column layout
C_BIAS = [0, 256, 512]
C_G = [768, 1024, 1280]
C_BE = [1536, 1792, 2048]
C_FCB = 2304
C_INV = 2560
C_EPS = 2624
C_R256 = 2625
C_IOTAC = 2626
C_SEL = 2688
CW = 3712
JH = 264                     # h_ext width: 256 h + 4 al_s + 4 al_d


def _interleave_perm():
    """col 4c+h <- 64h+c for H=4, C=64."""
    p = np.zeros(256, np.int64)
    for h in range(HEADS):
        for c in range(HIDDEN):
            p[4 * c + h] = 64 * h + c
    return p


def _prep_weights(ws):
    """Fold attention vectors into W, apply channel interleave permutations.

    Returns per-layer W_ext [256, 264] plus permuted ln/bias vectors.
    """
    perm = _interleave_perm()
    out = {}
    in_perm = np.arange(256)
    for l in range(3):
        W = ws[f"w{l}"].astype(np.float64)
        a_s = ws[f"as{l}"].astype(np.float64)
        a_d = ws[f"ad{l}"].astype(np.float64)
        heads = HEADS if l < 2 else 1
        outc = HIDDEN if l < 2 else EMBED
        # WA[cin, h] = sum_c W[cin, h*outc + c] * a[h, c]
        Wr = W.reshape(256, heads, outc)
        wa_s = np.einsum("khc,hc->kh", Wr, a_s)
        wa_d = np.einsum("khc,hc->kh", Wr, a_d)
        Wc = W.copy()
        out_perm = np.arange(256)
        if l < 2:
            Wc = Wc[:, perm]
            out_perm = perm
        Wx = np.zeros((256, JH), np.float64)
        Wx[:, :256] = Wc
        Wx[:, 256 : 256 + heads] = wa_s
        Wx[:, 260 : 260 + heads] = wa_d
        Wx = Wx[in_perm, :]  # permute input rows to match previous layer's interleave
        out[f"wext{l}"] = Wx.astype(np.float32)
        out[f"bias{l}"] = ws[f"b{l}"][out_perm].astype(np.float32)
        out[f"g{l}"] = ws[f"g{l}"][out_perm].astype(np.float32)
        out[f"be{l}"] = ws[f"be{l}"][out_perm].astype(np.float32)
        in_perm = out_perm
    out["fc_w"] = ws["fc_w"].astype(np.float32)  # layer-2 out is plain
    out["fc_b"] = ws["fc_b"].astype(np.float32)
    return out


def _prep_edges(edge_index, batch):
    """Partition/sort/pad edge structure (vectorized). Returns shared block
    structure and per-core index arrays."""
    src = np.concatenate([edge_index[0].astype(np.int64), np.arange(N)])
    dst = np.concatenate([edge_index[1].astype(np.int64), np.arange(N)])
    # map src node id to padded table row id
    spad = (src // PARTN) * PADN + (src % PARTN)
    core = dst // PARTN

    per_core = []
    nblk = np.zeros((NCORES, NW, 2), np.int64)
    for c in range(NCORES):
        m = core == c
        s, d = spad[m], dst[m] - c * PARTN
        w = d // DW
        half = (s >= SPLIT).astype(np.int64)
        order = np.lexsort((s, half, w))
        s, d, w, half = s[order], d[order], w[order], half[order]
        per_core.append((s, d, w, half))
        cnts = np.bincount(w * 2 + half, minlength=NW * 2).reshape(NW, 2)
        nblk[c] = (cnts + 127) // 128

    NBLH = nblk.max(axis=0)  # [NW, 2]

    # shared block list: per window, lo blocks then hi blocks
    blocks = []  # (window, half, first, last)
    blk_start = np.zeros((NW, 2), np.int64)
    pos = 0
    for wi in range(NW):
        tot = int(NBLH[wi, 0] + NBLH[wi, 1])
        blk_start[wi, 0] = pos
        for b in range(int(NBLH[wi, 0])):
            blocks.append((wi, 0, b == 0, b + 1 == tot))
        pos += int(NBLH[wi, 0])
        blk_start[wi, 1] = pos
        for b in range(int(NBLH[wi, 1])):
            blocks.append(
                (wi, 1, NBLH[wi, 0] == 0 and b == 0, b + 1 + NBLH[wi, 0] == tot)
            )
        pos += int(NBLH[wi, 1])
    NBLK = len(blocks)

    sidx = np.zeros((NCORES, 128, NBLK * 8), np.int16)
    didx = np.zeros((NCORES, 128, NBLK * 8), np.int16)
    dloc = np.full((NCORES, 128, NBLK), -1.0, np.float32)

    for c in range(NCORES):
        s, d, w, half = per_core[c]
        n = len(s)
        g = w * 2 + half  # sorted group key
        starts = np.r_[0, np.flatnonzero(np.diff(g)) + 1]
        gstart = starts[np.searchsorted(g[starts], g)]
        pos_in = np.arange(n) - gstart
        blk = blk_start[w, half] + pos_in // 128
        p = pos_in % 128
        rows = p % 16
        cols = blk * 8 + p // 16
        sc = np.zeros((128, NBLK * 8), np.int16)
        dc = np.zeros((128, NBLK * 8), np.int16)
        sc[rows, cols] = s - half * SPLIT
        dc[rows, cols] = d
        for k in range(1, 8):
            sc[16 * k : 16 * k + 16] = sc[:16]
            dc[16 * k : 16 * k + 16] = dc[:16]
        sidx[c] = sc
        didx[c] = dc
        dl = np.full((128, NBLK), -1.0, np.float32)
        dl[p, blk] = (d - w * DW).astype(np.float32)
        dloc[c] = dl

    # graph ids per node slot, and counts
    b64 = batch.astype(np.int64)
    nl = (np.arange(128)[:, None] + 128 * np.arange(NT)[None, :])  # [128, NT]
    gid = np.zeros((NCORES, 128, NT), np.float32)
    for c in range(NCORES):
        valid = nl < PARTN
        gv = b64[c * PARTN + np.minimum(nl, PARTN - 1)]
        gid[c] = np.where(valid, gv.astype(np.float32), -1.0)
    cnt = np.bincount(b64, minlength=NG).astype(np.float64)
    inv_cnt = (1.0 / np.maximum(cnt, 1.0)).astype(np.float32)

    return blocks, NBLK, sidx, didx, dloc, gid, inv_cnt


_PROGRAM_CACHE = {}
_LAST_RESULT = None


def _build_program(blocks, NBLK):
    nc = bacc.Bacc("TRN2", target_bir_lowering=False, debug=False, num_devices=NCORES)

    # ---- inputs ----
    xT0 = nc.dram_tensor("xT0", [128, 2, PADN], F32, kind="ExternalInput").ap()
    wext = [
        nc.dram_tensor(f"wext{l}", [256, JH], F32, kind="ExternalInput").ap()
        for l in range(3)
    ]
    fcw = nc.dram_tensor("fcw", [256, 256], F32, kind="ExternalInput").ap()
    sidx = nc.dram_tensor("sidx", [128, NBLK * 8], I16, kind="ExternalInput").ap()
    dloc = nc.dram_tensor("dloc", [128, NBLK], F32, kind="ExternalInput").ap()
    gid = nc.dram_tensor("gid", [128, NT], F32, kind="ExternalInput").ap()
    iota = nc.dram_tensor("iota", [128, 128], F32, kind="ExternalInput").ap()
    ident = nc.dram_tensor("ident", [128, 128], F32, kind="ExternalInput").ap()
    # consts: per layer bias/g/be (256 each), fcb 256, invcnt 64, scalars
    consts = nc.dram_tensor("consts", [128, CW], F32, kind="ExternalInput").ap()
    out_t = nc.dram_tensor("out", [NG, 256], F32, kind="ExternalOutput").ap()
    xdumps = (
        [
            nc.dram_tensor(f"xdump{l}", [128, NT, 256], F32, kind="ExternalOutput").ap()
            for l in range(3)
        ]
        if GAT_DEBUG
        else None
    )
    if GAT_DEBUG:
        aldump = nc.dram_tensor("aldump", [PADN, 8], F32, kind="ExternalOutput").ap()
        exdump = nc.dram_tensor("exdump", [128, CHUNK, 4], F32, kind="ExternalOutput").ap()
        nardump = nc.dram_tensor("nardump", [128, CHUNK, 4], F32, kind="ExternalOutput").ap()
        alddump = nc.dram_tensor("alddump", [128, CHUNK, 8], F32, kind="ExternalOutput").ap()
        gdump = nc.dram_tensor("gdump", [128, CHUNK, 16], F32, kind="ExternalOutput").ap()

    mm = mybir.AluOpType

    import contextlib

    with tile.TileContext(nc) as tc, contextlib.ExitStack() as _ctx:
        if True:
            cpool = _ctx.enter_context(tc.tile_pool(name="const", bufs=1))
            spool = _ctx.enter_context(tc.tile_pool(name="struct", bufs=1))
            wpool = _ctx.enter_context(tc.tile_pool(name="w", bufs=1))
            xtpool = _ctx.enter_context(tc.tile_pool(name="xt", bufs=1))
            xspool = _ctx.enter_context(tc.tile_pool(name="xs", bufs=1))
            epool = _ctx.enter_context(tc.tile_pool(name="evac", bufs=2))
            gpool = _ctx.enter_context(tc.tile_pool(name="gath", bufs=2))
            npool = _ctx.enter_context(tc.tile_pool(name="nar", bufs=2))
            scrpool = _ctx.enter_context(tc.tile_pool(name="scr", bufs=2))
            smpool = _ctx.enter_context(tc.tile_pool(name="small", bufs=2))
            phpool = _ctx.enter_context(tc.tile_pool(name="ph", bufs=1, space="PSUM"))
            paggpool = _ctx.enter_context(tc.tile_pool(name="pagg", bufs=2, space="PSUM"))
            ptpool = _ctx.enter_context(tc.tile_pool(name="pt", bufs=2, space="PSUM"))
            pdtpool = _ctx.enter_context(tc.tile_pool(name="pdt", bufs=1, space="PSUM"))
            paldpool = _ctx.enter_context(tc.tile_pool(name="pald", bufs=1, space="PSUM"))
            pfcpool = _ctx.enter_context(tc.tile_pool(name="pfc", bufs=1, space="PSUM"))
            dram = _ctx.enter_context(tc.tile_pool(name="dram", bufs=1, space="DRAM"))
            nc.gpsimd.load_library(library_config.mlp)

            # persistent SBUF loads
            iota_sb = cpool.tile([128, 128], F32, tag="iota")
            ident_sb = cpool.tile([128, 128], F32, tag="ident")
            consts_sb = cpool.tile([128, CW], F32, tag="consts")
            dloc_sb = spool.tile([128, NBLK], F32, tag="dloc")
            gid_sb = spool.tile([128, NT], F32, tag="gid")
            nc.sync.dma_start(iota_sb[:], iota[:])
            nc.sync.dma_start(ident_sb[:], ident[:])
            nc.sync.dma_start(consts_sb[:], consts[:])
            nc.sync.dma_start(dloc_sb[:], dloc[:])
            nc.sync.dma_start(gid_sb[:], gid[:])

            xT = xtpool.tile([128, 2, PADN], F32, tag="xT")
            nc.sync.dma_start(xT[:], xT0[:])

            x_stage = xspool.tile([128, NT, 256], F32, tag="xstage")

            # DRAM tiles
            tab_slice = dram.tile([PADN, TROW], TDT)
            al_slice = dram.tile([PADN, 64], F32)
            ar_in = dram.tile([NG, 256], F32)
            ar_out = dram.tile([NG, 256], F32, addr_space="Shared")

            tab_slice_v = tab_slice[:].rearrange("(t p) r -> p t r", p=128)
            al_slice_v = al_slice[:].rearrange("(t p) r -> p t r", p=128)

            # chunk plan: list of (c0, cb) over blocks
            chunks = []
            c0 = 0
            while c0 < NBLK:
                cb = min(CHUNK, NBLK - c0)
                chunks.append((c0, cb))
                c0 += cb

            def expand_ald(l, c0, cb, ALD, alw_sb):
                # per-edge al_dst via PE expansion instead of a gather:
                # dT = dloc chunk transposed; rep_b = ones x dT[b] (row
                # replicated to 128 partitions); cmpT = (rep == partition);
                # alD[:, b] = cmpT matmul with the window al values.
                dt_ps = pdtpool.tile([CHUNK, 128], F32, tag="dt", name=f"dt{l}_{c0}")
                nc.tensor.transpose(
                    dt_ps[0:cb, :], dloc_sb[:, c0 : c0 + cb], ident_sb[:]
                )
                dt_sb = npool.tile([CHUNK, 128], F32, tag="dtsb", name=f"dts{l}_{c0}")
                nc.vector.tensor_copy(dt_sb[0:cb, :], dt_ps[0:cb, :])
                iotac_b = consts_sb[:, C_IOTAC : C_IOTAC + 1].broadcast_to([128, 128])
                ald_ps = paldpool.tile(
                    [128, 4 * CHUNK], F32, tag="aldps", name=f"alp{l}_{c0}"
                )
                rep_tiles = {}
                for j in range((cb + 3) // 4):
                    rep_tiles[j] = ptpool.tile(
                        [128, 4, 128], F32, tag="pt", name=f"rep{l}_{c0}_{j}"
                    )
                    jb = min(4, cb - 4 * j)
                    for q in range(jb):
                        b = 4 * j + q
                        nc.tensor.matmul(
                            rep_tiles[j][:, q, :],
                            consts_sb[0:cb, C_SEL + 128 * b : C_SEL + 128 * (b + 1)],
                            dt_sb[0:cb, :],
                            start=(q == 0),
                            stop=(q == jb - 1),
                            skip_group_check=True,
                        )
                for b in range(cb):
                    cmpt = npool.tile(
                        [128, 128], F32, tag="cmpt", name=f"ct{l}_{c0}_{b}"
                    )
                    nc.vector.tensor_tensor(
                        cmpt[:], rep_tiles[b // 4][:, b % 4, :], iotac_b, mm.is_equal
                    )
                    wi_b = blocks[c0 + b][0]
                    nc.tensor.matmul(
                        ald_ps[:, 4 * b : 4 * b + 4],
                        cmpt[:],
                        alw_sb[:, wi_b, :],
                        start=(b == 0),
                        stop=(b == cb - 1),
                        skip_group_check=True,
                    )
                nc.vector.tensor_copy(
                    ALD[:, 0:cb, 0:4],
                    ald_ps[:, 0 : 4 * cb].rearrange("p (b a) -> p b a", a=4),
                )

            for l in range(GAT_LAYERS):
                nh = HEADS if l < 2 else 1
                tab_full = dram.tile(
                    [NPAD, TROW], TDT, addr_space="Shared", name=f"tab_full{l}"
                )
                wsb = wpool.tile([128, 2, JH], F32, tag="w")
                nc.sync.dma_start(wsb[:], wext[l].rearrange("(k p) j -> p k j", p=128))

                # ---- phase 1: h_ext slice + table/al writes ----
                for nb in range(NT):
                    ph = phpool.tile([128, JH], F32, tag="ph")
                    for kc in range(2):
                        nc.tensor.matmul(
                            ph[:],
                            xT[:, kc, nb * 128 : (nb + 1) * 128],
                            wsb[:, kc, :],
                            start=(kc == 0),
                            stop=(kc == 1),
                        )
                    ev = epool.tile([128, TROW], TDT, tag="ev")
                    # h channels (cast to table dtype)
                    nc.vector.tensor_copy(ev[:, 0:256], ph[:, 0:256])
                    if USE_BF16:
                        # al_src kept as raw f32 bytes inside the bf16 row
                        als_f32 = ev[:, 256:264].bitcast(F32)
                        nc.vector.tensor_copy(als_f32, ph[:, 256:260])
                    else:
                        nc.vector.tensor_copy(ev[:, 256:260], ph[:, 256:260])
                    alv = epool.tile([128, 8], F32, tag="alv")
                    nc.vector.tensor_copy(alv[:], ph[:, 256:264])
                    nc.sync.dma_start(tab_slice_v[:, nb, :], ev[:])
                    nc.sync.dma_start(al_slice_v[:, nb, 0:8], alv[:])

                alw_sb = wpool.tile([128, NT, 4], F32, tag="alw")
                nc.sync.dma_start(
                    alw_sb[:],
                    al_slice[:, 4:8].rearrange("(t p) r -> p t r", p=128),
                )
                # ---- phase 2: allgather table ----
                if GAT_NO_AG:
                    nc.sync.dma_start(tab_full[0:PADN, :], tab_slice[:])
                else:
                    nc.gpsimd.collective_compute(
                        "AllGather",
                        mm.bypass,
                        replica_groups=[list(range(NCORES))],
                        ins=[tab_slice.opt()],
                        outs=[tab_full.opt()],
                    )
                tab_lo = tab_full[0:SPLIT, :]
                tab_hi = tab_full[SPLIT:NPAD, :]

                # ---- phase 3: gather + aggregate ----
                win_psum = {}
                for c0, cb in chunks:
                    G = gpool.tile([128, CHUNK, TROW], TDT, tag="G")
                    ALD = npool.tile([128, CHUNK, 4], F32, tag="ALD")
                    if GAT_NO_GATHER:
                        nc.vector.memset(G[:], 0.5)
                        nc.vector.memset(ALD[:], 0.25)
                    else:
                        sidx_sb = npool.tile([128, CHUNK * 8], I16, tag="sidxc")
                        nc.sync.dma_start(
                            sidx_sb[:, 0 : cb * 8], sidx[:, c0 * 8 : (c0 + cb) * 8]
                        )
                        # gather runs grouped by src half
                        r0 = 0
                        while r0 < cb:
                            hf = blocks[c0 + r0][1]
                            r1 = r0
                            while r1 < cb and blocks[c0 + r1][1] == hf:
                                r1 += 1
                            nrun = (r1 - r0) * 128
                            nc.gpsimd.dma_gather(
                                G[:, r0:r1, :],
                                tab_lo if hf == 0 else tab_hi,
                                sidx_sb[:, r0 * 8 : r1 * 8],
                                nrun,
                                nrun,
                                TROW,
                            )
                            r0 = r1
                    expand_ald(l, c0, cb, ALD, alw_sb)
                    # narrow attention math
                    nar = npool.tile([128, CHUNK, 4], F32, tag="nar")
                    if USE_BF16:
                        als_ap = G[:, 0:cb, 256:264].bitcast(F32)[:, :, 0:nh]
                    else:
                        als_ap = G[:, 0:cb, 256 : 256 + nh]
                    nc.vector.tensor_tensor(
                        nar[:, 0:cb, 0:nh], als_ap, ALD[:, 0:cb, 0:nh], mm.add
                    )
                    lr = npool.tile([128, CHUNK, 4], F32, tag="lr")
                    nc.vector.tensor_scalar_mul(lr[:, 0:cb, 0:nh], nar[:, 0:cb, 0:nh], NEG)
                    nc.vector.tensor_tensor(
                        nar[:, 0:cb, 0:nh], nar[:, 0:cb, 0:nh], lr[:, 0:cb, 0:nh], mm.max
                    )
                    ex = npool.tile([128, CHUNK, 4], TDT, tag="ex")
                    nc.scalar.activation(
                        ex[:, 0:cb, 0:nh],
                        nar[:, 0:cb, 0:nh],
                        mybir.ActivationFunctionType.Exp,
                    )
                    if GAT_DEBUG and l == 0 and c0 == 0:
                        nc.sync.dma_start(exdump[:, 0:cb, 0:nh], ex[:, 0:cb, 0:nh])
                        nc.sync.dma_start(nardump[:, 0:cb, 0:nh], nar[:, 0:cb, 0:nh])
                        nc.sync.dma_start(alddump[:, :, 0:4], ALD[:, :, 0:4])
                        if USE_BF16:
                            nc.sync.dma_start(
                                gdump[:, 0:cb, 0:4], G[:, 0:cb, 256:264].bitcast(F32)
                            )
                        else:
                            nc.sync.dma_start(gdump[:], G[:, :, 256:272])
                    cmp = npool.tile([128, CHUNK, DW], TDT, tag="cmp")
                    dl_b = dloc_sb[:, c0 : c0 + cb].unsqueeze(2).broadcast_to(
                        [128, cb, DW]
                    )
                    io_b = iota_sb[:].unsqueeze(1).broadcast_to([128, cb, DW])
                    nc.vector.tensor_tensor(cmp[:, 0:cb, :], dl_b, io_b, mm.is_equal)
                    # prescale h channels by ex
                    if l < 2:
                        g_v = G[:, 0:cb, 0:256].rearrange("p b (c h) -> p b c h", h=4)
                        ex_b = ex[:, 0:cb, :].unsqueeze(2).broadcast_to([128, cb, 64, 4])
                        nc.vector.tensor_tensor(g_v, g_v, ex_b, mm.mult)
                    else:
                        ex_b = ex[:, 0:cb, 0:1].broadcast_to([128, cb, 256])
                        nc.vector.tensor_tensor(
                            G[:, 0:cb, 0:256], G[:, 0:cb, 0:256], ex_b, mm.mult
                        )

                    for b in range(cb):
                        if GAT_NO_MM:
                            break
                        wi, hf, first, last = blocks[c0 + b]
                        if first:
                            win_psum[wi] = paggpool.tile([128, 260], F32, tag="pagg", name=f"pagg{wi}")
                        pw = win_psum[wi]
                        # start=True clears the whole bank: issue it exactly
                        # once (den MM of the first block), all else accumulate
                        nc.tensor.matmul(
                            pw[:, 256 : 256 + nh],
                            cmp[:, b, :],
                            ex[:, b, 0:nh],
                            start=first,
                            stop=last,
                            skip_group_check=True,
                        )
                        nc.tensor.matmul(
                            pw[:, 0:256],
                            cmp[:, b, :],
                            G[:, b, 0:256],
                            start=False,
                            stop=last,
                            skip_group_check=True,
                        )
                        if last:
                            den = smpool.tile([128, 4], F32, tag="den")
                            nc.vector.tensor_scalar_max(
                                den[:, 0:nh], pw[:, 256 : 256 + nh], 1e-30
                            )
                            rden = smpool.tile([128, 4], F32, tag="rden")
                            nc.vector.reciprocal(rden[:, 0:nh], den[:, 0:nh])
                            if l < 2:
                                x_v = x_stage[:, wi, :].rearrange(
                                    "p (c h) -> p c h", h=4
                                )
                                p_v = pw[:, 0:256].rearrange("p (c h) -> p c h", h=4)
                                rd_b = rden[:].unsqueeze(1).broadcast_to([128, 64, 4])
                                nc.vector.tensor_tensor(x_v, p_v, rd_b, mm.mult)
                            else:
                                rd_b = rden[:, 0:1].broadcast_to([128, 256])
                                nc.vector.tensor_tensor(
                                    x_stage[:, wi, :], pw[:, 0:256], rd_b, mm.mult
                                )
                            del win_psum[wi]

                if GAT_NO_MM:
                    nc.vector.memset(x_stage[:], 0.125)
                # ---- phase 4: bias + layernorm + elu (batched) ----
                def cvec(col):
                    return (
                        consts_sb[:, col : col + 256]
                        .unsqueeze(1)
                        .broadcast_to([128, NT, 256])
                    )

                xs = x_stage[:]
                nc.vector.tensor_tensor(xs, xs, cvec(C_BIAS[l]), mm.add)
                msum = smpool.tile([128, NT], F32, tag="msum")
                nc.vector.tensor_reduce(msum[:], xs, mybir.AxisListType.X, mm.add)
                mu = smpool.tile([128, NT], F32, tag="mu")
                nc.vector.tensor_scalar_mul(mu[:], msum[:], 1.0 / 256.0)
                mu_b = mu[:].unsqueeze(2).broadcast_to([128, NT, 256])
                nc.vector.tensor_tensor(xs, xs, mu_b, mm.subtract)
                ss = smpool.tile([128, NT], F32, tag="ss")
                for t0 in range(0, NT, 4):
                    t1 = min(t0 + 4, NT)
                    sq = scrpool.tile([128, 4, 256], F32, tag="sq")
                    nc.vector.tensor_tensor(
                        sq[:, 0 : t1 - t0, :],
                        x_stage[:, t0:t1, :],
                        x_stage[:, t0:t1, :],
                        mm.mult,
                    )
                    nc.vector.tensor_reduce(
                        ss[:, t0:t1],
                        sq[:, 0 : t1 - t0, :],
                        mybir.AxisListType.X,
                        mm.add,
                    )
                sd = smpool.tile([128, NT], F32, tag="sd")
                nc.scalar.activation(
                    sd[:],
                    ss[:],
                    mybir.ActivationFunctionType.Sqrt,
                    bias=consts_sb[:, C_EPS : C_EPS + 1],
                    scale=consts_sb[:, C_R256 : C_R256 + 1],
                )
                rstd = smpool.tile([128, NT], F32, tag="rstd")
                nc.vector.reciprocal(rstd[:], sd[:])
                rstd_b = rstd[:].unsqueeze(2).broadcast_to([128, NT, 256])
                nc.vector.tensor_tensor(xs, xs, rstd_b, mm.mult)
                nc.vector.tensor_tensor(xs, xs, cvec(C_G[l]), mm.mult)
                nc.vector.tensor_tensor(xs, xs, cvec(C_BE[l]), mm.add)
                # elu(x) = max(x,0) + min(e^x,1) - 1
                for t0 in range(0, NT, 4):
                    t1 = min(t0 + 4, NT)
                    ee = scrpool.tile([128, 4, 256], F32, tag="sq")
                    nc.scalar.activation(
                        ee[:, 0 : t1 - t0, :],
                        x_stage[:, t0:t1, :],
                        mybir.ActivationFunctionType.Exp,
                    )
                    nc.vector.tensor_scalar(
                        ee[:, 0 : t1 - t0, :],
                        ee[:, 0 : t1 - t0, :],
                        1.0,
                        -1.0,
                        mm.min,
                        mm.add,
                    )
                    nc.vector.tensor_scalar_max(
                        x_stage[:, t0:t1, :], x_stage[:, t0:t1, :], 0.0
                    )
                    nc.vector.tensor_tensor(
                        x_stage[:, t0:t1, :],
                        x_stage[:, t0:t1, :],
                        ee[:, 0 : t1 - t0, :],
                        mm.add,
                    )

                if GAT_DEBUG:
                    nc.sync.dma_start(xdumps[l][:], x_stage[:])
                    if l == 0:
                        nc.sync.dma_start(
                            aldump[:], al_slice[:, 0:8]
                        )
                # ---- phase 5: transpose x for next layer ----
                if l < 2:
                    for nb in range(NT):
                        for kc in range(2):
                            pt = ptpool.tile([128, 128], F32, tag="pt")
                            nc.tensor.transpose(
                                pt[:],
                                x_stage[:, nb, kc * 128 : (kc + 1) * 128],
                                ident_sb[:],
                            )
                            nc.vector.tensor_copy(
                                xT[:, kc, nb * 128 : (nb + 1) * 128], pt[:]
                            )

            if GAT_LAYERS == 0:
                nc.vector.memset(x_stage[:], 0.125)
            # ---- pooling ----
            if GAT_NO_POOL:
                cmpg = None
            cmpg = scrpool.tile([128, NT, NG], F32, tag="cmpg", bufs=1)
            gid_b = gid_sb[:].unsqueeze(2).broadcast_to([128, NT, NG])
            io64_b = iota_sb[:, 0:NG].unsqueeze(1).broadcast_to([128, NT, NG])
            nc.vector.tensor_tensor(cmpg[:], gid_b, io64_b, mm.is_equal)
            pp = pfcpool.tile([NG, 256], F32, tag="pfc")
            if GAT_NO_POOL:
                nc.vector.memset(pp[:], 0.0)
            for b in range(NT) if not GAT_NO_POOL else []:
                nc.tensor.matmul(
                    pp[:],
                    cmpg[:, b, :],
                    x_stage[:, b, :],
                    start=(b == 0),
                    stop=(b == NT - 1),
                )
            pooled = smpool.tile([NG, 256], F32, tag="pooled")
            nc.vector.tensor_copy(pooled[:], pp[:])
            nc.sync.dma_start(ar_in[:], pooled[:])
            if GAT_NO_AR:
                nc.sync.dma_start(ar_out[:], ar_in[:])
            else:
                nc.gpsimd.collective_compute(
                    "AllReduce",
                    mm.add,
                    replica_groups=[list(range(NCORES))],
                    ins=[ar_in.opt()],
                    outs=[ar_out.opt()],
                )
            pooled2 = smpool.tile([NG, 256], F32, tag="pooled2")
            nc.sync.dma_start(pooled2[:], ar_out[:])
            if not GAT_NO_INV:
                nc.vector.tensor_scalar_mul(
                    pooled2[:], pooled2[:], consts_sb[0:NG, C_INV : C_INV + 1]
                )
            # fc
            if GAT_NO_FC:
                nc.sync.dma_start(out_t[:], pooled2[:])
            fcw_sb = wpool.tile([128, 2, 256], F32, tag="fcw")
            nc.sync.dma_start(fcw_sb[:], fcw.rearrange("(k p) j -> p k j", p=128))
            poolT = smpool.tile([128, 2, NG], F32, tag="poolT")
            for kc in range(2):
                pt = ptpool.tile([128, 128], F32, tag="pt")
                nc.tensor.transpose(
                    pt[0:128, 0:NG],
                    pooled2[:, kc * 128 : (kc + 1) * 128],
                    ident_sb[0:NG, 0:NG],
                )
                nc.vector.tensor_copy(poolT[:, kc, :], pt[0:128, 0:NG])
            pfc = pfcpool.tile([NG, 256], F32, tag="pfc")
            for kc in range(2):
                nc.tensor.matmul(
                    pfc[:],
                    poolT[:, kc, :],
                    fcw_sb[:, kc, :],
                    start=(kc == 0),
                    stop=(kc == 1),
                )
            ores = smpool.tile([NG, 256], F32, tag="ores")
            fcb_b = consts_sb[0:NG, C_FCB : C_FCB + 256]
            nc.vector.tensor_tensor(ores[:], pfc[:], fcb_b, mm.add)
            nc.vector.tensor_scalar_max(ores[:], ores[:], 0.0)
            if not GAT_NO_FC:
                nc.sync.dma_start(out_t[:], ores[:])

    nc.compile()
    return nc


def kernel(**inputs):
    x = np.asarray(inputs["x"], np.float32)
    edge_index = np.asarray(inputs["edge_index"])
    batch = np.asarray(inputs["batch"])

    blocks, NBLK, sidx, didx, dloc, gid, inv_cnt = _prep_edges(edge_index, batch)
    wp = _prep_weights(inputs)

    key = (NBLK, USE_BF16, CHUNK, GAT_DEBUG, GAT_LAYERS, GAT_NO_AG, GAT_NO_GATHER, GAT_NO_MM, GAT_NO_AR, GAT_NO_POOL, GAT_NO_INV, GAT_NO_FC, tuple(b for b in blocks[:8]))
    if key not in _PROGRAM_CACHE:
        _PROGRAM_CACHE[key] = _build_program(blocks, NBLK)
    nc = _PROGRAM_CACHE[key]

    iota = np.broadcast_to(np.arange(128, dtype=np.float32), (128, 128)).copy()
    ident = np.eye(128, dtype=np.float32)
    consts = np.zeros((128, CW), np.float32)
    for l in range(3):
        consts[:, C_BIAS[l] : C_BIAS[l] + 256] = wp[f"bias{l}"][None, :]
        consts[:, C_G[l] : C_G[l] + 256] = wp[f"g{l}"][None, :]
        consts[:, C_BE[l] : C_BE[l] + 256] = wp[f"be{l}"][None, :]
    consts[:, C_FCB : C_FCB + 256] = wp["fc_b"][None, :]
    consts[:NG, C_INV] = inv_cnt
    consts[NG:, C_INV] = 1.0
    consts[:, C_EPS] = EPS
    consts[:, C_R256] = 1.0 / 256.0
    consts[:, C_IOTAC] = np.arange(128, dtype=np.float32)
    for q in range(8):
        consts[q, C_SEL + 128 * q : C_SEL + 128 * (q + 1)] = 1.0

    in_maps = []
    for c in range(NCORES):
        xs = np.zeros((PADN, 256), np.float32)
        xs[:PARTN] = x[c * PARTN : (c + 1) * PARTN]
        xT0 = np.ascontiguousarray(
            xs.T.reshape(2, 128, PADN).transpose(1, 0, 2)
        )  # [128, 2, PADN]; xT0[p,k,n] = xs[n, 128k+p]
        in_maps.append(
            {
                "xT0": xT0,
                "wext0": wp["wext0"],
                "wext1": wp["wext1"],
                "wext2": wp["wext2"],
                "fcw": wp["fc_w"],
                "sidx": sidx[c],
                "dloc": dloc[c],
                "gid": gid[c],
                "iota": iota,
                "ident": ident,
                "consts": consts,
            }
        )

    global _LAST_RESULT
    res = run_bass_kernel_spmd(nc, in_maps, core_ids=list(range(NCORES)), trace=False)
    _LAST_RESULT = res
    return res.results[0]["out"]



# revision 33
# speedup vs baseline: 1.4846x; 1.4846x over previous
"""3-layer GAT + global-mean-pool + FC on 8 Trainium2 NeuronCores.

Graph/data-parallel per the sharding hint: nodes and their incident (dst)
edges are sharded across 8 cores; weights are replicated.

v2 pipeline (vs. the phase-serial baseline):
  - Per 128-dst window, aggregation finalize immediately runs bias+LN+ELU,
    transposes the result, computes the NEXT layer's h tile and writes it to
    the next layer's table slice. No batched LN phase, no separate
    transpose/phase-1 passes.
  - The per-layer table is split in two row ranges (A: local rows <3072,
    B: rest). Each half is AllGathered separately as soon as its tiles are
    written, so the collectives hide under gather processing of the previous
    half/layer. The A/B split also keeps gather indices within int16.
  - Self-loop edges are removed from the gather stream entirely: their
    contribution exp(lrelu(als+ald))*h (plus the denominator term) is
    precomputed per node at h-time (slh) and folded into the window PSUM
    with one identity matmul.
  - One fused matmul per 128-edge block: the exp'd logits are written into
    the gathered rows' spare columns so messages and softmax denominators
    accumulate in a single [128x128]x[128x260] matmul.
  - bf16 everywhere on the PE (x, W, tables, cmp masks); attention
    pointwise math on ACT (Lrelu/Exp) and batched DVE ops per chunk.
"""

import os
import sys

for _p in ("/opt/trn_rl_repo", "/opt/pypackages"):
    if _p not in sys.path:
        sys.path.append(_p)

import numpy as np
import ml_dtypes

import concourse.bass as bass
import concourse.bacc as bacc
import concourse.tile as tile
import concourse.mybir as mybir
from concourse import library_config
from concourse.bass_utils import run_bass_kernel_spmd

# ---- problem constants (hardcoded per spec) ----
N = 50000
NCORES = 8
PARTN = N // NCORES          # 6250 real nodes per core
EMBED = 256
HIDDEN = 64
HEADS = 4
NG = 64                      # graphs
EPS = 1e-5
NEG = 0.2
DW = 128                     # dst window
NT = 49                      # node tiles per core
PADN = NT * 128              # 6272 padded local rows
ASPL = 3072                  # local rows < ASPL go to table A
BROW = PADN - ASPL           # 3200 rows per core in table B
NA = NCORES * ASPL           # 24576 global A rows
NB = NCORES * BROW           # 25600 global B rows
ATILES = ASPL // 128         # 24
CH = int(os.environ.get("GAT_CHUNK", "24"))
GCALL = int(os.environ.get("GAT_GCALL", "8"))  # max blocks per dma_gather call
GAT_NOACC = os.environ.get("GAT_NOACC", "0") == "1"
GAT_AGLATE = os.environ.get("GAT_AGLATE", "0") == "1"
SUB = 4                      # ALD expansion subchunk (1 PSUM bank)
TRIG_A_W = 29                # window whose finalize emits next layer's AG-A

GAT_LAYERS = int(os.environ.get("GAT_LAYERS", "3"))

F32 = mybir.dt.float32
BF16 = mybir.dt.bfloat16
I16 = mybir.dt.int16
BF = ml_dtypes.bfloat16

TROW = 384                   # table row: 256 h bf16 + als as raw f32 + pad
JH = 264                     # h_ext width: 256 h + 4 al_s + 4 al_d

# consts tile column layout (f32)
C_BIAS = [0, 256, 512]
C_G = [768, 1024, 1280]
C_BE = [1536, 1792, 2048]
C_FCB = 2304
C_INV = 2560
C_EPS = 2624
C_R256 = 2625
C_IOTAC = 2626
C_SEL = 2688
CW = 3712


def _interleave_perm():
    p = np.zeros(256, np.int64)
    for h in range(HEADS):
        for c in range(HIDDEN):
            p[4 * c + h] = 64 * h + c
    return p


def _prep_weights(ws):
    """Fold attention vectors into W, apply channel interleave permutations."""
    perm = _interleave_perm()
    out = {}
    in_perm = np.arange(256)
    for l in range(3):
        W = np.asarray(ws[f"w{l}"], np.float64)
        a_s = np.asarray(ws[f"as{l}"], np.float64)
        a_d = np.asarray(ws[f"ad{l}"], np.float64)
        heads = HEADS if l < 2 else 1
        outc = HIDDEN if l < 2 else EMBED
        Wr = W.reshape(256, heads, outc)
        wa_s = np.einsum("khc,hc->kh", Wr, a_s)
        wa_d = np.einsum("khc,hc->kh", Wr, a_d)
        Wc = W.copy()
        out_perm = np.arange(256)
        if l < 2:
            Wc = Wc[:, perm]
            out_perm = perm
        Wx = np.zeros((256, JH), np.float64)
        Wx[:, :256] = Wc
        Wx[:, 256 : 256 + heads] = wa_s
        Wx[:, 260 : 260 + heads] = wa_d
        Wx = Wx[in_perm, :]
        out[f"wext{l}"] = Wx.astype(BF)
        out[f"bias{l}"] = np.asarray(ws[f"b{l}"], np.float64)[out_perm].astype(np.float32)
        out[f"g{l}"] = np.asarray(ws[f"g{l}"], np.float64)[out_perm].astype(np.float32)
        out[f"be{l}"] = np.asarray(ws[f"be{l}"], np.float64)[out_perm].astype(np.float32)
        in_perm = out_perm
    out["fc_w"] = np.asarray(ws["fc_w"], np.float32)
    out["fc_b"] = np.asarray(ws["fc_b"], np.float32)
    return out


def _prep_edges(edge_index, batch):
    """Partition/sort/pad edge structure. Self-loops are NOT added (handled
    analytically on device). Returns shared block structure + per-core data."""
    src = np.asarray(edge_index[0], np.int64)
    dst = np.asarray(edge_index[1], np.int64)
    # Random (i,i) edges stay in the stream; only the reference's appended
    # self-loop per node is handled analytically (slh fold) on device.
    lr = src % PARTN
    piece = (lr >= ASPL).astype(np.int64)
    srow = np.where(piece == 0, (src // PARTN) * ASPL + lr,
                    (src // PARTN) * BROW + (lr - ASPL))
    core = dst // PARTN

    per_core = []
    nblk = np.zeros((NCORES, NT, 2), np.int64)
    for c in range(NCORES):
        m = core == c
        s, d, h = srow[m], dst[m] - c * PARTN, piece[m]
        w = d // DW
        order = np.lexsort((s, h, w))
        s, d, w, h = s[order], d[order], w[order], h[order]
        per_core.append((s, d, w, h))
        cnts = np.bincount(w * 2 + h, minlength=NT * 2).reshape(NT, 2)
        nblk[c] = (cnts + 127) // 128

    NBLH = nblk.max(axis=0)  # [NT, 2]

    blocks = []  # (window, piece, first_of_window, last_of_window)
    blk_start = np.zeros((NT, 2), np.int64)
    pos = 0
    for wi in range(NT):
        tot = int(NBLH[wi, 0] + NBLH[wi, 1])
        blk_start[wi, 0] = pos
        for b in range(int(NBLH[wi, 0])):
            blocks.append((wi, 0, b == 0, b + 1 == tot))
        pos += int(NBLH[wi, 0])
        blk_start[wi, 1] = pos
        for b in range(int(NBLH[wi, 1])):
            blocks.append(
                (wi, 1, NBLH[wi, 0] == 0 and b == 0, b + 1 + NBLH[wi, 0] == tot)
            )
        pos += int(NBLH[wi, 1])
    NBLK = len(blocks)

    sidx = np.zeros((NCORES, 128, NBLK * 8), np.int16)
    dloc = np.full((NCORES, 128, NBLK), -1.0, np.float32)

    for c in range(NCORES):
        s, d, w, h = per_core[c]
        n = len(s)
        g = w * 2 + h
        starts = np.r_[0, np.flatnonzero(np.diff(g)) + 1]
        gstart = starts[np.searchsorted(g[starts], g)]
        pos_in = np.arange(n) - gstart
        blk = blk_start[w, h] + pos_in // 128
        p = pos_in % 128
        rows = p % 16
        cols = blk * 8 + p // 16
        sc = np.zeros((128, NBLK * 8), np.int16)
        sc[rows, cols] = s
        for k in range(1, 8):
            sc[16 * k : 16 * k + 16] = sc[:16]
        sidx[c] = sc
        dl = np.full((128, NBLK), -1.0, np.float32)
        dl[p, blk] = (d - w * DW).astype(np.float32)
        dloc[c] = dl

    b64 = np.asarray(batch, np.int64)
    nl = np.arange(128)[:, None] + 128 * np.arange(NT)[None, :]
    gid = np.zeros((NCORES, 128, NT), np.float32)
    for c in range(NCORES):
        valid = nl < PARTN
        gv = b64[c * PARTN + np.minimum(nl, PARTN - 1)]
        gid[c] = np.where(valid, gv.astype(np.float32), -1.0)
    cnt = np.bincount(b64, minlength=NG).astype(np.float64)
    inv_cnt = (1.0 / np.maximum(cnt, 1.0)).astype(np.float32)

    return blocks, NBLK, sidx, dloc, gid, inv_cnt


_PROGRAM_CACHE = {}
_LAST_RESULT = None


def _build_program(blocks, NBLK):
    nc = bacc.Bacc("TRN2", target_bir_lowering=False, debug=False, num_devices=NCORES)
    mm = mybir.AluOpType
    AF = mybir.ActivationFunctionType

    # ---- inputs ----
    xT0 = nc.dram_tensor("xT0", [128, 2, PADN], BF16, kind="ExternalInput").ap()
    wext = [
        nc.dram_tensor(f"wext{l}", [256, JH], BF16, kind="ExternalInput").ap()
        for l in range(3)
    ]
    fcw = nc.dram_tensor("fcw", [256, 256], F32, kind="ExternalInput").ap()
    sidx_d = nc.dram_tensor("sidx", [128, NBLK * 8], I16, kind="ExternalInput").ap()
    dloc_d = nc.dram_tensor("dloc", [128, NBLK], F32, kind="ExternalInput").ap()
    gid_d = nc.dram_tensor("gid", [128, NT], F32, kind="ExternalInput").ap()
    iota_d = nc.dram_tensor("iota", [128, 128], F32, kind="ExternalInput").ap()
    ident_d = nc.dram_tensor("ident", [128, 128], F32, kind="ExternalInput").ap()
    consts_d = nc.dram_tensor("consts", [128, CW], F32, kind="ExternalInput").ap()
    out_t = nc.dram_tensor("out", [NG, 256], F32, kind="ExternalOutput").ap()

    # chunk plan
    chunks = []
    c0 = 0
    while c0 < NBLK:
        cb = min(CH, NBLK - c0)
        chunks.append((c0, cb))
        c0 += cb

    import contextlib

    with tile.TileContext(nc) as tc, contextlib.ExitStack() as _ctx:
        cpool = _ctx.enter_context(tc.tile_pool(name="const", bufs=1))
        spool = _ctx.enter_context(tc.tile_pool(name="struct", bufs=1))
        wpool = _ctx.enter_context(tc.tile_pool(name="w", bufs=1))
        slpool = _ctx.enter_context(tc.tile_pool(name="slh", bufs=2))
        gpool = _ctx.enter_context(tc.tile_pool(name="gath", bufs=2))
        cmpool = _ctx.enter_context(tc.tile_pool(name="cmp", bufs=2))
        npool = _ctx.enter_context(tc.tile_pool(name="nar", bufs=2))
        epool = _ctx.enter_context(tc.tile_pool(name="evac", bufs=2))
        ypool = _ctx.enter_context(tc.tile_pool(name="y", bufs=2))
        xpool = _ctx.enter_context(tc.tile_pool(name="xtw", bufs=2))
        smpool = _ctx.enter_context(tc.tile_pool(name="small", bufs=2))
        pct = _ctx.enter_context(tc.tile_pool(name="pct", bufs=1, space="PSUM"))
        prag = _ctx.enter_context(tc.tile_pool(name="prag", bufs=2, space="PSUM"))
        pald = _ctx.enter_context(tc.tile_pool(name="pald", bufs=1, space="PSUM"))
        pdt = _ctx.enter_context(tc.tile_pool(name="pdt", bufs=1, space="PSUM"))
        pfin = _ctx.enter_context(tc.tile_pool(name="pfin", bufs=2, space="PSUM"))
        dram = _ctx.enter_context(tc.tile_pool(name="dram", bufs=1, space="DRAM"))
        nc.gpsimd.load_library(library_config.mlp)

        # persistent SBUF
        iota_sb = cpool.tile([128, 128], F32, tag="iota")
        ident_sb = cpool.tile([128, 128], F32, tag="ident")
        consts_sb = cpool.tile([128, CW], F32, tag="consts")
        nc.sync.dma_start(iota_sb[:], iota_d[:])
        nc.sync.dma_start(ident_sb[:], ident_d[:])
        nc.sync.dma_start(consts_sb[:], consts_d[:])
        iota_bf = cpool.tile([128, 128], BF16, tag="iotabf")
        ident_bf = cpool.tile([128, 128], BF16, tag="identbf")
        nc.vector.tensor_copy(iota_bf[:], iota_sb[:])
        nc.vector.tensor_copy(ident_bf[:], ident_sb[:])
        dloc_f = spool.tile([128, NBLK], F32, tag="dlocf")
        dloc_bf = spool.tile([128, NBLK], BF16, tag="dlocbf")
        sidx_sb = spool.tile([128, NBLK * 8], I16, tag="sidx")
        gid_sb = spool.tile([128, NT], F32, tag="gid")
        nc.sync.dma_start(dloc_f[:], dloc_d[:])
        nc.sync.dma_start(sidx_sb[:], sidx_d[:])
        nc.sync.dma_start(gid_sb[:], gid_d[:])
        nc.vector.tensor_copy(dloc_bf[:], dloc_f[:])

        wsb = []
        for l in range(3):
            wt = wpool.tile([128, 2, JH], BF16, tag=f"w{l}")
            nc.sync.dma_start(wt[:], wext[l].rearrange("(k p) j -> p k j", p=128))
            wsb.append(wt)

        # per-layer DRAM tables
        tsA, tsB, tabA, tabB = [], [], [], []
        for l in range(GAT_LAYERS):
            tsA.append(dram.tile([ASPL, TROW], BF16, name=f"tsA{l}"))
            tsB.append(dram.tile([BROW, TROW], BF16, name=f"tsB{l}"))
            tabA.append(dram.tile([NA, TROW], BF16, addr_space="Shared", name=f"tabA{l}"))
            tabB.append(dram.tile([NB, TROW], BF16, addr_space="Shared", name=f"tabB{l}"))
        ar_in = dram.tile([NG, 256], F32)
        ar_out = dram.tile([NG, 256], F32, addr_space="Shared")

        def ag(l, which):
            src = tsA[l] if which == 0 else tsB[l]
            dst = tabA[l] if which == 0 else tabB[l]
            nc.gpsimd.collective_compute(
                "AllGather",
                mm.bypass,
                replica_groups=[list(range(NCORES))],
                ins=[src.opt()],
                outs=[dst.opt()],
            )

        # slh / alw ping-pong (producer: layer l-1 loop; consumer: layer l)
        def slh_tile():
            return slpool.tile([128, NT, 260], BF16, tag="slh", name="slh")

        def alw_tile():
            return slpool.tile([128, NT, 4], BF16, tag="alw", name="alw")

        pooled_ps = {}

        def emit_h(l_next, w, xtw, slh_nx, alw_nx):
            """Compute h^{l_next} for window w from transposed x (xtw),
            stash slh/alw, write the table row tile."""
            nh2 = HEADS if l_next < 2 else 1
            fin = pfin.tile([128, JH], F32, tag="fin")
            ph = fin[:]
            for kc in range(2):
                nc.tensor.matmul(
                    ph,
                    xtw[:, kc, :],
                    wsb[l_next][:, kc, :],
                    start=(kc == 0),
                    stop=(kc == 1),
                )
            # self-loop factor: sl = exp(lrelu(als+ald))
            als8 = smpool.tile([128, 8], F32, tag="als8")
            nc.vector.tensor_copy(als8[:], ph[:, 256:264])
            sl = smpool.tile([128, 4], F32, tag="sl")
            nc.vector.tensor_tensor(
                sl[:, 0:nh2], als8[:, 0:nh2], als8[:, 4 : 4 + nh2], mm.add
            )
            slr = smpool.tile([128, 4], F32, tag="slr")
            nc.vector.tensor_scalar_mul(slr[:, 0:nh2], sl[:, 0:nh2], NEG)
            nc.vector.tensor_tensor(
                sl[:, 0:nh2], sl[:, 0:nh2], slr[:, 0:nh2], mm.max
            )
            exf = smpool.tile([128, 4], F32, tag="exf")
            nc.scalar.activation(exf[:, 0:nh2], sl[:, 0:nh2], AF.Exp)
            nc.vector.tensor_copy(slh_nx[:, w, 256 : 256 + nh2], exf[:, 0:nh2])
            if nh2 == 4:
                slh_v = slh_nx[:, w, 0:256].rearrange("p (c h) -> p c h", h=4)
                ph_v = ph[:, 0:256].rearrange("p (c h) -> p c h", h=4)
                ex_b = exf[:].unsqueeze(1).broadcast_to([128, 64, 4])
                nc.vector.tensor_tensor(slh_v, ph_v, ex_b, mm.mult)
            else:
                ex_b = exf[:, 0:1].broadcast_to([128, 256])
                nc.vector.tensor_tensor(
                    slh_nx[:, w, 0:256], ph[:, 0:256], ex_b, mm.mult
                )
            nc.vector.tensor_copy(alw_nx[:, w, 0:nh2], ph[:, 260 : 260 + nh2])
            ev = epool.tile([128, TROW], BF16, tag="ev")
            nc.vector.tensor_copy(ev[:, 0:256], ph[:, 0:256])
            nc.vector.tensor_copy(ev[:, 256:264].bitcast(F32), ph[:, 256:260])
            nc.vector.memset(ev[:, 264:TROW], 0.0)
            if w < ATILES:
                dst = tsA[l_next][:].rearrange("(t p) r -> p t r", p=128)[:, w, :]
            else:
                dst = tsB[l_next][:].rearrange("(t p) r -> p t r", p=128)[:, w - ATILES, :]
            nc.sync.dma_start(dst, ev[:])

        def finalize_window(l, w, pw, slh_nx, alw_nx):
            nh = HEADS if l < 2 else 1
            den = smpool.tile([128, 4], F32, tag="den")
            nc.vector.tensor_scalar_max(den[:, 0:nh], pw[:, 256 : 256 + nh], 1e-30)
            rden = smpool.tile([128, 4], F32, tag="rden")
            nc.vector.reciprocal(rden[:, 0:nh], den[:, 0:nh])
            y = ypool.tile([128, 256], F32, tag="y")
            if l < 2:
                y_v = y[:].rearrange("p (c h) -> p c h", h=4)
                p_v = pw[:, 0:256].rearrange("p (c h) -> p c h", h=4)
                rd_b = rden[:].unsqueeze(1).broadcast_to([128, 64, 4])
                nc.vector.tensor_tensor(y_v, p_v, rd_b, mm.mult)
            else:
                rd_b = rden[:, 0:1].broadcast_to([128, 256])
                nc.vector.tensor_tensor(y[:], pw[:, 0:256], rd_b, mm.mult)
            bias_b = consts_sb[:, C_BIAS[l] : C_BIAS[l] + 256]
            nc.vector.tensor_tensor(y[:], y[:], bias_b, mm.add)
            # LN: mean via DVE reduce, E[x^2] via ACT square+accum
            s1 = smpool.tile([128, 1], F32, tag="s1")
            nc.vector.tensor_reduce(s1[:], y[:], mybir.AxisListType.X, mm.add)
            ysq = ypool.tile([128, 256], F32, tag="ysq")
            ss = smpool.tile([128, 1], F32, tag="ss")
            if GAT_NOACC:
                nc.vector.tensor_tensor(ysq[:], y[:], y[:], mm.mult)
                nc.vector.tensor_reduce(ss[:], ysq[:], mybir.AxisListType.X, mm.add)
            else:
                nc.scalar.activation(ysq[:], y[:], AF.Square, accum_out=ss[:])
            mu = smpool.tile([128, 1], F32, tag="mu")
            nc.vector.tensor_scalar_mul(mu[:], s1[:], 1.0 / 256.0)
            var = smpool.tile([128, 1], F32, tag="var")
            nc.vector.tensor_scalar_mul(var[:], ss[:], 1.0 / 256.0)
            mu2 = smpool.tile([128, 1], F32, tag="mu2")
            nc.vector.tensor_tensor(mu2[:], mu[:], mu[:], mm.mult)
            nc.vector.tensor_tensor(var[:], var[:], mu2[:], mm.subtract)
            sd = smpool.tile([128, 1], F32, tag="sd")
            nc.scalar.activation(
                sd[:], var[:], AF.Sqrt, bias=consts_sb[:, C_EPS : C_EPS + 1]
            )
            rstd = smpool.tile([128, 1], F32, tag="rstd")
            nc.vector.reciprocal(rstd[:], sd[:])
            nmr = smpool.tile([128, 1], F32, tag="nmr")
            nc.vector.tensor_tensor(nmr[:], mu[:], rstd[:], mm.mult)
            nc.vector.tensor_scalar_mul(nmr[:], nmr[:], -1.0)
            # y1 = y*rstd - mu*rstd
            nc.scalar.activation(
                y[:], y[:], AF.Identity, bias=nmr[:], scale=rstd[:]
            )
            nc.vector.tensor_tensor(
                y[:], y[:], consts_sb[:, C_G[l] : C_G[l] + 256], mm.mult
            )
            nc.vector.tensor_tensor(
                y[:], y[:], consts_sb[:, C_BE[l] : C_BE[l] + 256], mm.add
            )
            # ELU: max(y,0) + min(exp(y),1) - 1
            ee = ypool.tile([128, 256], F32, tag="ee")
            nc.scalar.activation(ee[:], y[:], AF.Exp)
            nc.vector.tensor_scalar_min(ee[:], ee[:], 1.0)
            nc.vector.tensor_scalar_max(y[:], y[:], 0.0)
            nc.vector.scalar_tensor_tensor(y[:], y[:], -1.0, ee[:], mm.add, mm.add)

            if l + 1 < GAT_LAYERS:
                xtw = xpool.tile([128, 2, 128], BF16, tag="xtw")
                for kc in range(2):
                    pt = pdt.tile([128, 128], F32, tag="pt")
                    nc.tensor.transpose(pt[:], y[:, kc * 128 : (kc + 1) * 128], ident_sb[:])
                    nc.vector.tensor_copy(xtw[:, kc, :], pt[:])
                emit_h(l + 1, w, xtw, slh_nx, alw_nx)
                if w == TRIG_A_W and not GAT_AGLATE:
                    ag(l + 1, 0)
            if l == GAT_LAYERS - 1:
                cmpg = smpool.tile([128, NG], F32, tag="cmpg")
                gid_b = gid_sb[:, w : w + 1].broadcast_to([128, NG])
                nc.vector.tensor_tensor(cmpg[:], gid_b, iota_sb[:, 0:NG], mm.is_equal)
                cmbf = smpool.tile([128, NG], BF16, tag="cmbf")
                nc.vector.tensor_copy(cmbf[:], cmpg[:])
                ybf = ypool.tile([128, 256], BF16, tag="ybf")
                nc.vector.tensor_copy(ybf[:], y[:])
                pp = pooled_ps["pp"]
                nc.tensor.matmul(
                    pp[:],
                    cmbf[:],
                    ybf[:],
                    start=(w == 0),
                    stop=(w == NT - 1),
                    skip_group_check=True,
                )

        # ---- prologue: table(0) from input x ----
        slh_cur = slh_tile()
        alw_cur = alw_tile()
        for w in range(NT):
            xtw = xpool.tile([128, 2, 128], BF16, tag="xtw")
            nc.sync.dma_start(xtw[:], xT0[:, :, w * 128 : (w + 1) * 128])
            emit_h(0, w, xtw, slh_cur, alw_cur)
            if w == ATILES - 1 and not GAT_AGLATE:
                ag(0, 0)
        if GAT_AGLATE:
            ag(0, 0)
        ag(0, 1)

        # ---- layer loops ----
        for l in range(GAT_LAYERS):
            nh = HEADS if l < 2 else 1
            if l + 1 < GAT_LAYERS:
                slh_nx = slh_tile()
                alw_nx = alw_tile()
            else:
                slh_nx = alw_nx = None
            if l == GAT_LAYERS - 1:
                pooled_ps["pp"] = prag.tile(
                    [NG, 256], F32, tag="pp", bufs=1, name="pp"
                )
            win_psum = {}
            for ci, (c0, cb) in enumerate(chunks):
                G = gpool.tile([128, CH, TROW], BF16, tag="G")
                # gather calls per piece-run
                r0 = 0
                while r0 < cb:
                    pc = blocks[c0 + r0][1]
                    r1 = r0
                    while (
                        r1 < cb
                        and blocks[c0 + r1][1] == pc
                        and r1 - r0 < GCALL
                    ):
                        r1 += 1
                    nrun = (r1 - r0) * 128
                    nc.gpsimd.dma_gather(
                        G[:, r0:r1, :],
                        (tabA[l] if pc == 0 else tabB[l])[:],
                        sidx_sb[:, (c0 + r0) * 8 : (c0 + r1) * 8],
                        nrun,
                        nrun,
                        TROW,
                    )
                    r0 = r1
                # cmp masks (batched)
                cmp = cmpool.tile([128, CH, DW], BF16, tag="cmp")
                dl_b = (
                    dloc_bf[:, c0 : c0 + cb].unsqueeze(2).broadcast_to([128, cb, DW])
                )
                io_b = iota_bf[:].unsqueeze(1).broadcast_to([128, cb, DW])
                nc.vector.tensor_tensor(cmp[:, 0:cb, :], io_b, dl_b, mm.is_equal)
                # ALD expansion: cmpT via PE transpose of cmp, then matmul vs alw
                ald_ps = pald.tile([128, 4 * CH], F32, tag="ald")
                for s0 in range(0, cb, SUB):
                    sbk = min(SUB, cb - s0)
                    ct_ps = pct.tile([128, SUB, 128], BF16, tag="ct")
                    for j in range(sbk):
                        nc.tensor.matmul(
                            ct_ps[:, j, :],
                            cmp[:, s0 + j, :],
                            ident_bf[:],
                            is_transpose=True,
                            start=(j == 0),
                            stop=(j == sbk - 1),
                            skip_group_check=True,
                        )
                    cmpt = npool.tile([128, SUB, 128], BF16, tag="cmpt")
                    nc.vector.tensor_copy(cmpt[:, 0:sbk, :], ct_ps[:, 0:sbk, :])
                    for j in range(sbk):
                        b = s0 + j
                        nc.tensor.matmul(
                            ald_ps[:, nh * b : nh * b + nh],
                            cmpt[:, j, :],
                            alw_cur[:, blocks[c0 + b][0], 0:nh],
                            start=(b == 0),
                            stop=(b == cb - 1),
                            skip_group_check=True,
                        )
                # attention pointwise (batched per chunk)
                nar = npool.tile([128, CH * 4], F32, tag="nar")
                nar_v = nar[:, 0 : nh * cb].rearrange("p (b a) -> p b a", a=nh)
                als_v = G[:, 0:cb, 256:264].bitcast(F32)[:, :, 0:nh]
                ald_v = ald_ps[:, 0 : nh * cb].rearrange("p (b a) -> p b a", a=nh)
                nc.vector.tensor_tensor(nar_v, als_v, ald_v, mm.add)
                lrt = npool.tile([128, CH * 4], F32, tag="lrt")
                nc.vector.tensor_scalar_mul(
                    lrt[:, 0 : nh * cb], nar[:, 0 : nh * cb], NEG
                )
                nc.vector.tensor_tensor(
                    nar[:, 0 : nh * cb], nar[:, 0 : nh * cb], lrt[:, 0 : nh * cb],
                    mm.max,
                )
                nc.scalar.activation(
                    G[:, 0:cb, 256 : 256 + nh], nar_v, AF.Exp
                )
                if nh == 1:
                    # replicate ex to cols 257:260 so prescale uses the fast
                    # 4-wide broadcast pattern
                    nc.vector.tensor_copy(
                        G[:, 0:cb, 257:260],
                        G[:, 0:cb, 256:257].broadcast_to([128, cb, 3]),
                    )
                # prescale h by ex
                g_v = G[:, 0:cb, 0:256].rearrange("p b (c h) -> p b c h", h=4)
                ex_b = (
                    G[:, 0:cb, 256:260].unsqueeze(2).broadcast_to([128, cb, 64, 4])
                )
                nc.vector.tensor_tensor(g_v, g_v, ex_b, mm.mult)
                # aggregation
                for b in range(cb):
                    wi, pc, first, last = blocks[c0 + b]
                    if first:
                        pw = prag.tile([128, 260], F32, tag="agg")
                        win_psum[wi] = pw
                        nc.tensor.matmul(
                            pw[:, 0 : 256 + nh],
                            ident_bf[:],
                            slh_cur[:, wi, 0 : 256 + nh],
                            start=True,
                            stop=False,
                            skip_group_check=True,
                        )
                    pw = win_psum[wi]
                    nc.tensor.matmul(
                        pw[:, 0 : 256 + nh],
                        cmp[:, b, :],
                        G[:, b, 0 : 256 + nh],
                        start=False,
                        stop=last,
                        skip_group_check=True,
                    )
                    if last:
                        finalize_window(l, wi, pw, slh_nx, alw_nx)
                        del win_psum[wi]
            if l + 1 < GAT_LAYERS:
                if GAT_AGLATE:
                    ag(l + 1, 0)
                ag(l + 1, 1)
            slh_cur, alw_cur = slh_nx, alw_nx

        # ---- tail: AllReduce pooled sums + FC ----
        pooled = smpool.tile([NG, 256], F32, tag="pooled")
        nc.vector.tensor_copy(pooled[:], pooled_ps["pp"][:])
        nc.sync.dma_start(ar_in[:], pooled[:])
        nc.gpsimd.collective_compute(
            "AllReduce",
            mm.add,
            replica_groups=[list(range(NCORES))],
            ins=[ar_in.opt()],
            outs=[ar_out.opt()],
        )
        pooled2 = smpool.tile([NG, 256], F32, tag="pooled2")
        nc.sync.dma_start(pooled2[:], ar_out[:])
        nc.vector.tensor_scalar_mul(
            pooled2[:], pooled2[:], consts_sb[0:NG, C_INV : C_INV + 1]
        )
        fcw_sb = wpool.tile([128, 2, 256], F32, tag="fcw")
        nc.sync.dma_start(fcw_sb[:], fcw.rearrange("(k p) j -> p k j", p=128))
        poolT = smpool.tile([128, 2, NG], F32, tag="poolT")
        for kc in range(2):
            pt2 = pdt.tile([128, 128], F32, tag="pt")
            nc.tensor.transpose(
                pt2[0:128, 0:NG],
                pooled2[:, kc * 128 : (kc + 1) * 128],
                ident_sb[0:NG, 0:NG],
            )
            nc.vector.tensor_copy(poolT[:, kc, :], pt2[0:128, 0:NG])
        pfc = prag.tile([NG, 256], F32, tag="pp", bufs=1)
        for kc in range(2):
            nc.tensor.matmul(
                pfc[:],
                poolT[:, kc, :],
                fcw_sb[:, kc, :],
                start=(kc == 0),
                stop=(kc == 1),
            )
        ores = smpool.tile([NG, 256], F32, tag="ores")
        fcb_b = consts_sb[0:NG, C_FCB : C_FCB + 256]
        nc.vector.tensor_tensor(ores[:], pfc[:], fcb_b, mm.add)
        nc.vector.tensor_scalar_max(ores[:], ores[:], 0.0)
        nc.sync.dma_start(out_t[:], ores[:])

    nc.compile()
    return nc


def kernel(**inputs):
    x = np.asarray(inputs["x"], np.float32)
    edge_index = np.asarray(inputs["edge_index"])
    batch = np.asarray(inputs["batch"])

    blocks, NBLK, sidx, dloc, gid, inv_cnt = _prep_edges(edge_index, batch)
    wp = _prep_weights(inputs)

    key = (NBLK, CH, GAT_LAYERS, tuple(blocks[:8]))
    if key not in _PROGRAM_CACHE:
        _PROGRAM_CACHE[key] = _build_program(blocks, NBLK)
    nc = _PROGRAM_CACHE[key]

    iota = np.broadcast_to(np.arange(128, dtype=np.float32), (128, 128)).copy()
    ident = np.eye(128, dtype=np.float32)
    consts = np.zeros((128, CW), np.float32)
    for l in range(3):
        consts[:, C_BIAS[l] : C_BIAS[l] + 256] = wp[f"bias{l}"][None, :]
        consts[:, C_G[l] : C_G[l] + 256] = wp[f"g{l}"][None, :]
        consts[:, C_BE[l] : C_BE[l] + 256] = wp[f"be{l}"][None, :]
    consts[:, C_FCB : C_FCB + 256] = wp["fc_b"][None, :]
    consts[:NG, C_INV] = inv_cnt
    consts[NG:, C_INV] = 1.0
    consts[:, C_EPS] = EPS
    consts[:, C_R256] = 1.0 / 256.0
    consts[:, C_IOTAC] = np.arange(128, dtype=np.float32)
    for q in range(8):
        consts[q, C_SEL + 128 * q : C_SEL + 128 * (q + 1)] = 1.0

    in_maps = []
    for c in range(NCORES):
        xs = np.zeros((PADN, 256), np.float32)
        xs[:PARTN] = x[c * PARTN : (c + 1) * PARTN]
        xT0 = np.ascontiguousarray(
            xs.T.reshape(2, 128, PADN).transpose(1, 0, 2)
        ).astype(BF)
        in_maps.append(
            {
                "xT0": xT0,
                "wext0": wp["wext0"],
                "wext1": wp["wext1"],
                "wext2": wp["wext2"],
                "fcw": wp["fc_w"],
                "sidx": sidx[c],
                "dloc": dloc[c],
                "gid": gid[c],
                "iota": iota,
                "ident": ident,
                "consts": consts,
            }
        )

    global _LAST_RESULT
    res = run_bass_kernel_spmd(nc, in_maps, core_ids=list(range(NCORES)), trace=False)
    _LAST_RESULT = res
    return res.results[0]["out"]


# revision 35
# speedup vs baseline: 1.7322x; 1.1668x over previous
"""3-layer GAT + global-mean-pool + FC on 8 Trainium2 NeuronCores.

Graph/data-parallel per the sharding hint: nodes and their incident (dst)
edges are sharded across 8 cores; weights are replicated.

v2 pipeline (vs. the phase-serial baseline):
  - Per 128-dst window, aggregation finalize immediately runs bias+LN+ELU,
    transposes the result, computes the NEXT layer's h tile and writes it to
    the next layer's table slice. No batched LN phase, no separate
    transpose/phase-1 passes.
  - The per-layer table is split in two row ranges (A: local rows <3072,
    B: rest). Each half is AllGathered separately as soon as its tiles are
    written, so the collectives hide under gather processing of the previous
    half/layer. The A/B split also keeps gather indices within int16.
  - Self-loop edges are removed from the gather stream entirely: their
    contribution exp(lrelu(als+ald))*h (plus the denominator term) is
    precomputed per node at h-time (slh) and folded into the window PSUM
    with one identity matmul.
  - One fused matmul per 128-edge block: the exp'd logits are written into
    the gathered rows' spare columns so messages and softmax denominators
    accumulate in a single [128x128]x[128x260] matmul.
  - bf16 everywhere on the PE (x, W, tables, cmp masks); attention
    pointwise math on ACT (Lrelu/Exp) and batched DVE ops per chunk.
"""

import os
import sys

for _p in ("/opt/trn_rl_repo", "/opt/pypackages"):
    if _p not in sys.path:
        sys.path.append(_p)

import numpy as np
import ml_dtypes

import concourse.bass as bass
import concourse.bacc as bacc
import concourse.tile as tile
import concourse.mybir as mybir
from concourse import library_config
from concourse.bass_utils import run_bass_kernel_spmd

# ---- problem constants (hardcoded per spec) ----
N = 50000
NCORES = 8
PARTN = N // NCORES          # 6250 real nodes per core
EMBED = 256
HIDDEN = 64
HEADS = 4
NG = 64                      # graphs
EPS = 1e-5
NEG = 0.2
DW = 128                     # dst window
NT = 49                      # node tiles per core
PADN = NT * 128              # 6272 padded local rows
ASPL = 3072                  # local rows < ASPL go to table A
BROW = PADN - ASPL           # 3200 rows per core in table B
NA = NCORES * ASPL           # 24576 global A rows
NB = NCORES * BROW           # 25600 global B rows
ATILES = ASPL // 128         # 24
CH = int(os.environ.get("GAT_CHUNK", "24"))
GCALL = int(os.environ.get("GAT_GCALL", "8"))  # max blocks per dma_gather call
GAT_NOACC = os.environ.get("GAT_NOACC", "0") == "1"
GAT_AGLATE = os.environ.get("GAT_AGLATE", "0") == "1"
SUB = 4                      # ALD expansion subchunk (1 PSUM bank)
TRIG_A_W = 29                # window whose finalize emits next layer's AG-A

GAT_LAYERS = int(os.environ.get("GAT_LAYERS", "3"))

F32 = mybir.dt.float32
BF16 = mybir.dt.bfloat16
I16 = mybir.dt.int16
BF = ml_dtypes.bfloat16

TROW = 384                   # table row: 256 h bf16 + als as raw f32 + pad
JH = 264                     # h_ext width: 256 h + 4 al_s + 4 al_d

# consts tile column layout (f32)
C_BIAS = [0, 256, 512]
C_G = [768, 1024, 1280]
C_BE = [1536, 1792, 2048]
C_FCB = 2304
C_INV = 2560
C_EPS = 2624
C_R256 = 2625
C_IOTAC = 2626
C_SEL = 2688
CW = 3712


def _interleave_perm():
    p = np.zeros(256, np.int64)
    for h in range(HEADS):
        for c in range(HIDDEN):
            p[4 * c + h] = 64 * h + c
    return p


def _prep_weights(ws):
    """Fold attention vectors into W, apply channel interleave permutations."""
    perm = _interleave_perm()
    out = {}
    in_perm = np.arange(256)
    for l in range(3):
        W = np.asarray(ws[f"w{l}"], np.float64)
        a_s = np.asarray(ws[f"as{l}"], np.float64)
        a_d = np.asarray(ws[f"ad{l}"], np.float64)
        heads = HEADS if l < 2 else 1
        outc = HIDDEN if l < 2 else EMBED
        Wr = W.reshape(256, heads, outc)
        wa_s = np.einsum("khc,hc->kh", Wr, a_s)
        wa_d = np.einsum("khc,hc->kh", Wr, a_d)
        Wc = W.copy()
        out_perm = np.arange(256)
        if l < 2:
            Wc = Wc[:, perm]
            out_perm = perm
        Wx = np.zeros((256, JH), np.float64)
        Wx[:, :256] = Wc
        Wx[:, 256 : 256 + heads] = wa_s
        Wx[:, 260 : 260 + heads] = wa_d
        Wx = Wx[in_perm, :]
        out[f"wext{l}"] = Wx.astype(BF)
        out[f"bias{l}"] = np.asarray(ws[f"b{l}"], np.float64)[out_perm].astype(np.float32)
        out[f"g{l}"] = np.asarray(ws[f"g{l}"], np.float64)[out_perm].astype(np.float32)
        out[f"be{l}"] = np.asarray(ws[f"be{l}"], np.float64)[out_perm].astype(np.float32)
        in_perm = out_perm
    out["fc_w"] = np.asarray(ws["fc_w"], np.float32)
    out["fc_b"] = np.asarray(ws["fc_b"], np.float32)
    return out


def _prep_edges(edge_index, batch):
    """Partition/sort/pad edge structure. Self-loops are NOT added (handled
    analytically on device). Returns shared block structure + per-core data."""
    src = np.asarray(edge_index[0], np.int64)
    dst = np.asarray(edge_index[1], np.int64)
    # Random (i,i) edges stay in the stream; only the reference's appended
    # self-loop per node is handled analytically (slh fold) on device.
    lr = src % PARTN
    piece = (lr >= ASPL).astype(np.int64)
    srow = np.where(piece == 0, (src // PARTN) * ASPL + lr,
                    (src // PARTN) * BROW + (lr - ASPL))
    core = dst // PARTN

    per_core = []
    nblk = np.zeros((NCORES, NT, 2), np.int64)
    for c in range(NCORES):
        m = core == c
        s, d, h = srow[m], dst[m] - c * PARTN, piece[m]
        w = d // DW
        order = np.lexsort((s, h, w))
        s, d, w, h = s[order], d[order], w[order], h[order]
        per_core.append((s, d, w, h))
        cnts = np.bincount(w * 2 + h, minlength=NT * 2).reshape(NT, 2)
        nblk[c] = (cnts + 127) // 128

    NBLH = nblk.max(axis=0)  # [NT, 2]

    blocks = []  # (window, piece, first_of_window, last_of_window)
    blk_start = np.zeros((NT, 2), np.int64)
    pos = 0
    for wi in range(NT):
        tot = int(NBLH[wi, 0] + NBLH[wi, 1])
        blk_start[wi, 0] = pos
        for b in range(int(NBLH[wi, 0])):
            blocks.append((wi, 0, b == 0, b + 1 == tot))
        pos += int(NBLH[wi, 0])
        blk_start[wi, 1] = pos
        for b in range(int(NBLH[wi, 1])):
            blocks.append(
                (wi, 1, NBLH[wi, 0] == 0 and b == 0, b + 1 + NBLH[wi, 0] == tot)
            )
        pos += int(NBLH[wi, 1])
    NBLK = len(blocks)

    sidx = np.zeros((NCORES, 128, NBLK * 8), np.int16)
    dloc = np.full((NCORES, 128, NBLK), -1.0, np.float32)

    for c in range(NCORES):
        s, d, w, h = per_core[c]
        n = len(s)
        g = w * 2 + h
        starts = np.r_[0, np.flatnonzero(np.diff(g)) + 1]
        gstart = starts[np.searchsorted(g[starts], g)]
        pos_in = np.arange(n) - gstart
        blk = blk_start[w, h] + pos_in // 128
        p = pos_in % 128
        rows = p % 16
        cols = blk * 8 + p // 16
        sc = np.zeros((128, NBLK * 8), np.int16)
        sc[rows, cols] = s
        for k in range(1, 8):
            sc[16 * k : 16 * k + 16] = sc[:16]
        sidx[c] = sc
        dl = np.full((128, NBLK), -1.0, np.float32)
        dl[p, blk] = (d - w * DW).astype(np.float32)
        dloc[c] = dl

    b64 = np.asarray(batch, np.int64)
    nl = np.arange(128)[:, None] + 128 * np.arange(NT)[None, :]
    gid = np.zeros((NCORES, 128, NT), np.float32)
    for c in range(NCORES):
        valid = nl < PARTN
        gv = b64[c * PARTN + np.minimum(nl, PARTN - 1)]
        gid[c] = np.where(valid, gv.astype(np.float32), -1.0)
    cnt = np.bincount(b64, minlength=NG).astype(np.float64)
    inv_cnt = (1.0 / np.maximum(cnt, 1.0)).astype(np.float32)

    return blocks, NBLK, sidx, dloc, gid, inv_cnt


_PROGRAM_CACHE = {}
_LAST_RESULT = None


NQUEUES = int(os.environ.get("GAT_NQ", "4"))


def _build_program(blocks, NBLK):
    nc = bacc.Bacc(
        "TRN2",
        target_bir_lowering=False,
        debug=False,
        num_devices=NCORES,
        num_swdge_queues=NQUEUES,
    )
    mm = mybir.AluOpType
    AF = mybir.ActivationFunctionType

    # ---- inputs ----
    xT0 = nc.dram_tensor("xT0", [128, 2, PADN], BF16, kind="ExternalInput").ap()
    wext = [
        nc.dram_tensor(f"wext{l}", [256, JH], BF16, kind="ExternalInput").ap()
        for l in range(3)
    ]
    fcw = nc.dram_tensor("fcw", [256, 256], F32, kind="ExternalInput").ap()
    sidx_d = nc.dram_tensor("sidx", [128, NBLK * 8], I16, kind="ExternalInput").ap()
    dloc_d = nc.dram_tensor("dloc", [128, NBLK], F32, kind="ExternalInput").ap()
    gid_d = nc.dram_tensor("gid", [128, NT], F32, kind="ExternalInput").ap()
    iota_d = nc.dram_tensor("iota", [128, 128], F32, kind="ExternalInput").ap()
    ident_d = nc.dram_tensor("ident", [128, 128], F32, kind="ExternalInput").ap()
    consts_d = nc.dram_tensor("consts", [128, CW], F32, kind="ExternalInput").ap()
    out_t = nc.dram_tensor("out", [NG, 256], F32, kind="ExternalOutput").ap()

    # chunk plan
    chunks = []
    c0 = 0
    while c0 < NBLK:
        cb = min(CH, NBLK - c0)
        chunks.append((c0, cb))
        c0 += cb

    import contextlib

    with tile.TileContext(nc) as tc, contextlib.ExitStack() as _ctx:
        cpool = _ctx.enter_context(tc.tile_pool(name="const", bufs=1))
        spool = _ctx.enter_context(tc.tile_pool(name="struct", bufs=1))
        wpool = _ctx.enter_context(tc.tile_pool(name="w", bufs=1))
        slpool = _ctx.enter_context(tc.tile_pool(name="slh", bufs=2))
        gpool = _ctx.enter_context(tc.tile_pool(name="gath", bufs=2))
        cmpool = _ctx.enter_context(tc.tile_pool(name="cmp", bufs=2))
        npool = _ctx.enter_context(tc.tile_pool(name="nar", bufs=2))
        epool = _ctx.enter_context(tc.tile_pool(name="evac", bufs=2))
        ypool = _ctx.enter_context(tc.tile_pool(name="y", bufs=2))
        xpool = _ctx.enter_context(tc.tile_pool(name="xtw", bufs=2))
        smpool = _ctx.enter_context(tc.tile_pool(name="small", bufs=2))
        pct = _ctx.enter_context(tc.tile_pool(name="pct", bufs=1, space="PSUM"))
        prag = _ctx.enter_context(tc.tile_pool(name="prag", bufs=2, space="PSUM"))
        pald = _ctx.enter_context(tc.tile_pool(name="pald", bufs=1, space="PSUM"))
        pdt = _ctx.enter_context(tc.tile_pool(name="pdt", bufs=1, space="PSUM"))
        pfin = _ctx.enter_context(tc.tile_pool(name="pfin", bufs=2, space="PSUM"))
        dram = _ctx.enter_context(tc.tile_pool(name="dram", bufs=1, space="DRAM"))
        nc.gpsimd.load_library(library_config.mlp)

        # persistent SBUF
        iota_sb = cpool.tile([128, 128], F32, tag="iota")
        ident_sb = cpool.tile([128, 128], F32, tag="ident")
        consts_sb = cpool.tile([128, CW], F32, tag="consts")
        nc.sync.dma_start(iota_sb[:], iota_d[:])
        nc.sync.dma_start(ident_sb[:], ident_d[:])
        nc.sync.dma_start(consts_sb[:], consts_d[:])
        iota_bf = cpool.tile([128, 128], BF16, tag="iotabf")
        ident_bf = cpool.tile([128, 128], BF16, tag="identbf")
        nc.vector.tensor_copy(iota_bf[:], iota_sb[:])
        nc.vector.tensor_copy(ident_bf[:], ident_sb[:])
        dloc_f = spool.tile([128, NBLK], F32, tag="dlocf")
        dloc_bf = spool.tile([128, NBLK], BF16, tag="dlocbf")
        sidx_sb = spool.tile([128, NBLK * 8], I16, tag="sidx")
        gid_sb = spool.tile([128, NT], F32, tag="gid")
        nc.sync.dma_start(dloc_f[:], dloc_d[:])
        nc.sync.dma_start(sidx_sb[:], sidx_d[:])
        nc.sync.dma_start(gid_sb[:], gid_d[:])
        nc.vector.tensor_copy(dloc_bf[:], dloc_f[:])

        wsb = []
        for l in range(3):
            wt = wpool.tile([128, 2, JH], BF16, tag=f"w{l}")
            nc.sync.dma_start(wt[:], wext[l].rearrange("(k p) j -> p k j", p=128))
            wsb.append(wt)

        # per-layer DRAM tables
        tsA, tsB, tabA, tabB = [], [], [], []
        for l in range(GAT_LAYERS):
            tsA.append(dram.tile([ASPL, TROW], BF16, name=f"tsA{l}"))
            tsB.append(dram.tile([BROW, TROW], BF16, name=f"tsB{l}"))
            tabA.append(dram.tile([NA, TROW], BF16, addr_space="Shared", name=f"tabA{l}"))
            tabB.append(dram.tile([NB, TROW], BF16, addr_space="Shared", name=f"tabB{l}"))
        ar_in = dram.tile([NG, 256], F32)
        ar_out = dram.tile([NG, 256], F32, addr_space="Shared")

        def ag(l, which):
            src = tsA[l] if which == 0 else tsB[l]
            dst = tabA[l] if which == 0 else tabB[l]
            nc.gpsimd.collective_compute(
                "AllGather",
                mm.bypass,
                replica_groups=[list(range(NCORES))],
                ins=[src.opt()],
                outs=[dst.opt()],
            )

        # slh / alw ping-pong (producer: layer l-1 loop; consumer: layer l)
        def slh_tile():
            return slpool.tile([128, NT, 260], BF16, tag="slh", name="slh")

        def alw_tile():
            return slpool.tile([128, NT, 4], BF16, tag="alw", name="alw")

        pooled_ps = {}

        def emit_h(l_next, w, xtw, slh_nx, alw_nx):
            """Compute h^{l_next} for window w from transposed x (xtw),
            stash slh/alw, write the table row tile."""
            nh2 = HEADS if l_next < 2 else 1
            fin = pfin.tile([128, JH], F32, tag="fin")
            ph = fin[:]
            for kc in range(2):
                nc.tensor.matmul(
                    ph,
                    xtw[:, kc, :],
                    wsb[l_next][:, kc, :],
                    start=(kc == 0),
                    stop=(kc == 1),
                )
            # self-loop factor: sl = exp(lrelu(als+ald))
            als8 = smpool.tile([128, 8], F32, tag="als8")
            nc.vector.tensor_copy(als8[:], ph[:, 256:264])
            sl = smpool.tile([128, 4], F32, tag="sl")
            nc.vector.tensor_tensor(
                sl[:, 0:nh2], als8[:, 0:nh2], als8[:, 4 : 4 + nh2], mm.add
            )
            slr = smpool.tile([128, 4], F32, tag="slr")
            nc.vector.tensor_scalar_mul(slr[:, 0:nh2], sl[:, 0:nh2], NEG)
            nc.vector.tensor_tensor(
                sl[:, 0:nh2], sl[:, 0:nh2], slr[:, 0:nh2], mm.max
            )
            exf = smpool.tile([128, 4], F32, tag="exf")
            nc.scalar.activation(exf[:, 0:nh2], sl[:, 0:nh2], AF.Exp)
            nc.vector.tensor_copy(slh_nx[:, w, 256 : 256 + nh2], exf[:, 0:nh2])
            if nh2 == 4:
                slh_v = slh_nx[:, w, 0:256].rearrange("p (c h) -> p c h", h=4)
                ph_v = ph[:, 0:256].rearrange("p (c h) -> p c h", h=4)
                ex_b = exf[:].unsqueeze(1).broadcast_to([128, 64, 4])
                nc.vector.tensor_tensor(slh_v, ph_v, ex_b, mm.mult)
            else:
                ex_b = exf[:, 0:1].broadcast_to([128, 256])
                nc.vector.tensor_tensor(
                    slh_nx[:, w, 0:256], ph[:, 0:256], ex_b, mm.mult
                )
            nc.vector.tensor_copy(alw_nx[:, w, 0:nh2], ph[:, 260 : 260 + nh2])
            ev = epool.tile([128, TROW], BF16, tag="ev")
            nc.vector.tensor_copy(ev[:, 0:256], ph[:, 0:256])
            nc.vector.tensor_copy(ev[:, 256:264].bitcast(F32), ph[:, 256:260])
            nc.vector.memset(ev[:, 264:TROW], 0.0)
            if w < ATILES:
                dst = tsA[l_next][:].rearrange("(t p) r -> p t r", p=128)[:, w, :]
            else:
                dst = tsB[l_next][:].rearrange("(t p) r -> p t r", p=128)[:, w - ATILES, :]
            nc.sync.dma_start(dst, ev[:])

        def finalize_window(l, w, pw, slh_nx, alw_nx):
            nh = HEADS if l < 2 else 1
            den = smpool.tile([128, 4], F32, tag="den")
            nc.vector.tensor_scalar_max(den[:, 0:nh], pw[:, 256 : 256 + nh], 1e-30)
            rden = smpool.tile([128, 4], F32, tag="rden")
            nc.vector.reciprocal(rden[:, 0:nh], den[:, 0:nh])
            y = ypool.tile([128, 256], F32, tag="y")
            if l < 2:
                y_v = y[:].rearrange("p (c h) -> p c h", h=4)
                p_v = pw[:, 0:256].rearrange("p (c h) -> p c h", h=4)
                rd_b = rden[:].unsqueeze(1).broadcast_to([128, 64, 4])
                nc.vector.tensor_tensor(y_v, p_v, rd_b, mm.mult)
            else:
                rd_b = rden[:, 0:1].broadcast_to([128, 256])
                nc.vector.tensor_tensor(y[:], pw[:, 0:256], rd_b, mm.mult)
            bias_b = consts_sb[:, C_BIAS[l] : C_BIAS[l] + 256]
            nc.vector.tensor_tensor(y[:], y[:], bias_b, mm.add)
            # LN: mean via DVE reduce, E[x^2] via ACT square+accum
            s1 = smpool.tile([128, 1], F32, tag="s1")
            nc.vector.tensor_reduce(s1[:], y[:], mybir.AxisListType.X, mm.add)
            ysq = ypool.tile([128, 256], F32, tag="ysq")
            ss = smpool.tile([128, 1], F32, tag="ss")
            if GAT_NOACC:
                nc.vector.tensor_tensor(ysq[:], y[:], y[:], mm.mult)
                nc.vector.tensor_reduce(ss[:], ysq[:], mybir.AxisListType.X, mm.add)
            else:
                nc.scalar.activation(ysq[:], y[:], AF.Square, accum_out=ss[:])
            mu = smpool.tile([128, 1], F32, tag="mu")
            nc.vector.tensor_scalar_mul(mu[:], s1[:], 1.0 / 256.0)
            var = smpool.tile([128, 1], F32, tag="var")
            nc.vector.tensor_scalar_mul(var[:], ss[:], 1.0 / 256.0)
            mu2 = smpool.tile([128, 1], F32, tag="mu2")
            nc.vector.tensor_tensor(mu2[:], mu[:], mu[:], mm.mult)
            nc.vector.tensor_tensor(var[:], var[:], mu2[:], mm.subtract)
            sd = smpool.tile([128, 1], F32, tag="sd")
            nc.scalar.activation(
                sd[:], var[:], AF.Sqrt, bias=consts_sb[:, C_EPS : C_EPS + 1]
            )
            rstd = smpool.tile([128, 1], F32, tag="rstd")
            nc.vector.reciprocal(rstd[:], sd[:])
            nmr = smpool.tile([128, 1], F32, tag="nmr")
            nc.vector.tensor_tensor(nmr[:], mu[:], rstd[:], mm.mult)
            nc.vector.tensor_scalar_mul(nmr[:], nmr[:], -1.0)
            # y1 = y*rstd - mu*rstd
            nc.scalar.activation(
                y[:], y[:], AF.Identity, bias=nmr[:], scale=rstd[:]
            )
            nc.vector.tensor_tensor(
                y[:], y[:], consts_sb[:, C_G[l] : C_G[l] + 256], mm.mult
            )
            nc.vector.tensor_tensor(
                y[:], y[:], consts_sb[:, C_BE[l] : C_BE[l] + 256], mm.add
            )
            # ELU: max(y,0) + min(exp(y),1) - 1
            ee = ypool.tile([128, 256], F32, tag="ee")
            nc.scalar.activation(ee[:], y[:], AF.Exp)
            nc.vector.tensor_scalar_min(ee[:], ee[:], 1.0)
            nc.vector.tensor_scalar_max(y[:], y[:], 0.0)
            nc.vector.scalar_tensor_tensor(y[:], y[:], -1.0, ee[:], mm.add, mm.add)

            if l + 1 < GAT_LAYERS:
                xtw = xpool.tile([128, 2, 128], BF16, tag="xtw")
                for kc in range(2):
                    pt = pdt.tile([128, 128], F32, tag="pt")
                    nc.tensor.transpose(pt[:], y[:, kc * 128 : (kc + 1) * 128], ident_sb[:])
                    nc.vector.tensor_copy(xtw[:, kc, :], pt[:])
                emit_h(l + 1, w, xtw, slh_nx, alw_nx)
                if w == TRIG_A_W and not GAT_AGLATE:
                    ag(l + 1, 0)
            if l == GAT_LAYERS - 1:
                cmpg = smpool.tile([128, NG], F32, tag="cmpg")
                gid_b = gid_sb[:, w : w + 1].broadcast_to([128, NG])
                nc.vector.tensor_tensor(cmpg[:], gid_b, iota_sb[:, 0:NG], mm.is_equal)
                cmbf = smpool.tile([128, NG], BF16, tag="cmbf")
                nc.vector.tensor_copy(cmbf[:], cmpg[:])
                ybf = ypool.tile([128, 256], BF16, tag="ybf")
                nc.vector.tensor_copy(ybf[:], y[:])
                pp = pooled_ps["pp"]
                nc.tensor.matmul(
                    pp[:],
                    cmbf[:],
                    ybf[:],
                    start=(w == 0),
                    stop=(w == NT - 1),
                    skip_group_check=True,
                )

        # ---- prologue: table(0) from input x ----
        slh_cur = slh_tile()
        alw_cur = alw_tile()
        for w in range(NT):
            xtw = xpool.tile([128, 2, 128], BF16, tag="xtw")
            nc.sync.dma_start(xtw[:], xT0[:, :, w * 128 : (w + 1) * 128])
            emit_h(0, w, xtw, slh_cur, alw_cur)
            if w == ATILES - 1 and not GAT_AGLATE:
                ag(0, 0)
        if GAT_AGLATE:
            ag(0, 0)
        ag(0, 1)

        # ---- layer loops ----
        for l in range(GAT_LAYERS):
            nh = HEADS if l < 2 else 1
            if l + 1 < GAT_LAYERS:
                slh_nx = slh_tile()
                alw_nx = alw_tile()
            else:
                slh_nx = alw_nx = None
            if l == GAT_LAYERS - 1:
                pooled_ps["pp"] = prag.tile(
                    [NG, 256], F32, tag="pp", bufs=1, name="pp"
                )
            win_psum = {}
            gq = [0]
            for ci, (c0, cb) in enumerate(chunks):
                G = gpool.tile([128, CH, TROW], BF16, tag="G")
                # gather calls per piece-run, round-robined over SWDGE queues
                r0 = 0
                while r0 < cb:
                    pc = blocks[c0 + r0][1]
                    r1 = r0
                    while (
                        r1 < cb
                        and blocks[c0 + r1][1] == pc
                        and r1 - r0 < GCALL
                    ):
                        r1 += 1
                    nrun = (r1 - r0) * 128
                    nc.gpsimd.dma_gather(
                        G[:, r0:r1, :],
                        (tabA[l] if pc == 0 else tabB[l])[:],
                        sidx_sb[:, (c0 + r0) * 8 : (c0 + r1) * 8],
                        nrun,
                        nrun,
                        TROW,
                        queue_num=gq[0] % NQUEUES,
                    )
                    gq[0] += 1
                    r0 = r1
                # cmp masks (batched)
                cmp = cmpool.tile([128, CH, DW], BF16, tag="cmp")
                dl_b = (
                    dloc_bf[:, c0 : c0 + cb].unsqueeze(2).broadcast_to([128, cb, DW])
                )
                io_b = iota_bf[:].unsqueeze(1).broadcast_to([128, cb, DW])
                nc.vector.tensor_tensor(cmp[:, 0:cb, :], io_b, dl_b, mm.is_equal)
                # ALD expansion: cmpT via PE transpose of cmp, then matmul vs alw
                ald_ps = pald.tile([128, 4 * CH], F32, tag="ald")
                for s0 in range(0, cb, SUB):
                    sbk = min(SUB, cb - s0)
                    ct_ps = pct.tile([128, SUB, 128], BF16, tag="ct")
                    for j in range(sbk):
                        nc.tensor.matmul(
                            ct_ps[:, j, :],
                            cmp[:, s0 + j, :],
                            ident_bf[:],
                            is_transpose=True,
                            start=(j == 0),
                            stop=(j == sbk - 1),
                            skip_group_check=True,
                        )
                    cmpt = npool.tile([128, SUB, 128], BF16, tag="cmpt")
                    nc.vector.tensor_copy(cmpt[:, 0:sbk, :], ct_ps[:, 0:sbk, :])
                    for j in range(sbk):
                        b = s0 + j
                        nc.tensor.matmul(
                            ald_ps[:, nh * b : nh * b + nh],
                            cmpt[:, j, :],
                            alw_cur[:, blocks[c0 + b][0], 0:nh],
                            start=(b == 0),
                            stop=(b == cb - 1),
                            skip_group_check=True,
                        )
                # attention pointwise (batched per chunk)
                nar = npool.tile([128, CH * 4], F32, tag="nar")
                nar_v = nar[:, 0 : nh * cb].rearrange("p (b a) -> p b a", a=nh)
                als_v = G[:, 0:cb, 256:264].bitcast(F32)[:, :, 0:nh]
                ald_v = ald_ps[:, 0 : nh * cb].rearrange("p (b a) -> p b a", a=nh)
                nc.vector.tensor_tensor(nar_v, als_v, ald_v, mm.add)
                lrt = npool.tile([128, CH * 4], F32, tag="lrt")
                nc.vector.tensor_scalar_mul(
                    lrt[:, 0 : nh * cb], nar[:, 0 : nh * cb], NEG
                )
                nc.vector.tensor_tensor(
                    nar[:, 0 : nh * cb], nar[:, 0 : nh * cb], lrt[:, 0 : nh * cb],
                    mm.max,
                )
                nc.scalar.activation(
                    G[:, 0:cb, 256 : 256 + nh], nar_v, AF.Exp
                )
                if nh == 1:
                    # replicate ex to cols 257:260 so prescale uses the fast
                    # 4-wide broadcast pattern
                    nc.vector.tensor_copy(
                        G[:, 0:cb, 257:260],
                        G[:, 0:cb, 256:257].broadcast_to([128, cb, 3]),
                    )
                # prescale h by ex
                g_v = G[:, 0:cb, 0:256].rearrange("p b (c h) -> p b c h", h=4)
                ex_b = (
                    G[:, 0:cb, 256:260].unsqueeze(2).broadcast_to([128, cb, 64, 4])
                )
                nc.vector.tensor_tensor(g_v, g_v, ex_b, mm.mult)
                # aggregation
                for b in range(cb):
                    wi, pc, first, last = blocks[c0 + b]
                    if first:
                        pw = prag.tile([128, 260], F32, tag="agg")
                        win_psum[wi] = pw
                        nc.tensor.matmul(
                            pw[:, 0 : 256 + nh],
                            ident_bf[:],
                            slh_cur[:, wi, 0 : 256 + nh],
                            start=True,
                            stop=False,
                            skip_group_check=True,
                        )
                    pw = win_psum[wi]
                    nc.tensor.matmul(
                        pw[:, 0 : 256 + nh],
                        cmp[:, b, :],
                        G[:, b, 0 : 256 + nh],
                        start=False,
                        stop=last,
                        skip_group_check=True,
                    )
                    if last:
                        finalize_window(l, wi, pw, slh_nx, alw_nx)
                        del win_psum[wi]
            if l + 1 < GAT_LAYERS:
                if GAT_AGLATE:
                    ag(l + 1, 0)
                ag(l + 1, 1)
            slh_cur, alw_cur = slh_nx, alw_nx

        # ---- tail: AllReduce pooled sums + FC ----
        pooled = smpool.tile([NG, 256], F32, tag="pooled")
        nc.vector.tensor_copy(pooled[:], pooled_ps["pp"][:])
        nc.sync.dma_start(ar_in[:], pooled[:])
        nc.gpsimd.collective_compute(
            "AllReduce",
            mm.add,
            replica_groups=[list(range(NCORES))],
            ins=[ar_in.opt()],
            outs=[ar_out.opt()],
        )
        pooled2 = smpool.tile([NG, 256], F32, tag="pooled2")
        nc.sync.dma_start(pooled2[:], ar_out[:])
        nc.vector.tensor_scalar_mul(
            pooled2[:], pooled2[:], consts_sb[0:NG, C_INV : C_INV + 1]
        )
        fcw_sb = wpool.tile([128, 2, 256], F32, tag="fcw")
        nc.sync.dma_start(fcw_sb[:], fcw.rearrange("(k p) j -> p k j", p=128))
        poolT = smpool.tile([128, 2, NG], F32, tag="poolT")
        for kc in range(2):
            pt2 = pdt.tile([128, 128], F32, tag="pt")
            nc.tensor.transpose(
                pt2[0:128, 0:NG],
                pooled2[:, kc * 128 : (kc + 1) * 128],
                ident_sb[0:NG, 0:NG],
            )
            nc.vector.tensor_copy(poolT[:, kc, :], pt2[0:128, 0:NG])
        pfc = prag.tile([NG, 256], F32, tag="pp", bufs=1)
        for kc in range(2):
            nc.tensor.matmul(
                pfc[:],
                poolT[:, kc, :],
                fcw_sb[:, kc, :],
                start=(kc == 0),
                stop=(kc == 1),
            )
        ores = smpool.tile([NG, 256], F32, tag="ores")
        fcb_b = consts_sb[0:NG, C_FCB : C_FCB + 256]
        nc.vector.tensor_tensor(ores[:], pfc[:], fcb_b, mm.add)
        nc.vector.tensor_scalar_max(ores[:], ores[:], 0.0)
        nc.sync.dma_start(out_t[:], ores[:])

    nc.compile()
    return nc


def kernel(**inputs):
    x = np.asarray(inputs["x"], np.float32)
    edge_index = np.asarray(inputs["edge_index"])
    batch = np.asarray(inputs["batch"])

    blocks, NBLK, sidx, dloc, gid, inv_cnt = _prep_edges(edge_index, batch)
    wp = _prep_weights(inputs)

    key = (NBLK, CH, GAT_LAYERS, tuple(blocks[:8]))
    if key not in _PROGRAM_CACHE:
        _PROGRAM_CACHE[key] = _build_program(blocks, NBLK)
    nc = _PROGRAM_CACHE[key]

    iota = np.broadcast_to(np.arange(128, dtype=np.float32), (128, 128)).copy()
    ident = np.eye(128, dtype=np.float32)
    consts = np.zeros((128, CW), np.float32)
    for l in range(3):
        consts[:, C_BIAS[l] : C_BIAS[l] + 256] = wp[f"bias{l}"][None, :]
        consts[:, C_G[l] : C_G[l] + 256] = wp[f"g{l}"][None, :]
        consts[:, C_BE[l] : C_BE[l] + 256] = wp[f"be{l}"][None, :]
    consts[:, C_FCB : C_FCB + 256] = wp["fc_b"][None, :]
    consts[:NG, C_INV] = inv_cnt
    consts[NG:, C_INV] = 1.0
    consts[:, C_EPS] = EPS
    consts[:, C_R256] = 1.0 / 256.0
    consts[:, C_IOTAC] = np.arange(128, dtype=np.float32)
    for q in range(8):
        consts[q, C_SEL + 128 * q : C_SEL + 128 * (q + 1)] = 1.0

    in_maps = []
    for c in range(NCORES):
        xs = np.zeros((PADN, 256), np.float32)
        xs[:PARTN] = x[c * PARTN : (c + 1) * PARTN]
        xT0 = np.ascontiguousarray(
            xs.T.reshape(2, 128, PADN).transpose(1, 0, 2)
        ).astype(BF)
        in_maps.append(
            {
                "xT0": xT0,
                "wext0": wp["wext0"],
                "wext1": wp["wext1"],
                "wext2": wp["wext2"],
                "fcw": wp["fc_w"],
                "sidx": sidx[c],
                "dloc": dloc[c],
                "gid": gid[c],
                "iota": iota,
                "ident": ident,
                "consts": consts,
            }
        )

    global _LAST_RESULT
    res = run_bass_kernel_spmd(nc, in_maps, core_ids=list(range(NCORES)), trace=False)
    _LAST_RESULT = res
    return res.results[0]["out"]


# revision 44
# speedup vs baseline: 1.9410x; 1.1205x over previous
"""3-layer GAT + global-mean-pool + FC on 8 Trainium2 NeuronCores.

Graph/data-parallel per the sharding hint: nodes and their incident (dst)
edges are sharded across 8 cores; weights are replicated.

v2 pipeline (vs. the phase-serial baseline):
  - Per 128-dst window, aggregation finalize immediately runs bias+LN+ELU,
    transposes the result, computes the NEXT layer's h tile and writes it to
    the next layer's table slice. No batched LN phase, no separate
    transpose/phase-1 passes.
  - The per-layer table is split in two row ranges (A: local rows <3072,
    B: rest). Each half is AllGathered separately as soon as its tiles are
    written, so the collectives hide under gather processing of the previous
    half/layer. The A/B split also keeps gather indices within int16.
  - Self-loop edges are removed from the gather stream entirely: their
    contribution exp(lrelu(als+ald))*h (plus the denominator term) is
    precomputed per node at h-time (slh) and folded into the window PSUM
    with one identity matmul.
  - One fused matmul per 128-edge block: the exp'd logits are written into
    the gathered rows' spare columns so messages and softmax denominators
    accumulate in a single [128x128]x[128x260] matmul.
  - bf16 everywhere on the PE (x, W, tables, cmp masks); attention
    pointwise math on ACT (Lrelu/Exp) and batched DVE ops per chunk.
"""

import os
import sys

for _p in ("/opt/trn_rl_repo", "/opt/pypackages"):
    if _p not in sys.path:
        sys.path.append(_p)

import numpy as np
import ml_dtypes

import concourse.bass as bass
import concourse.bacc as bacc
import concourse.tile as tile
import concourse.mybir as mybir
from concourse import library_config
from concourse.bass_utils import run_bass_kernel_spmd

# ---- problem constants (hardcoded per spec) ----
N = 50000
NCORES = 8
PARTN = N // NCORES          # 6250 real nodes per core
EMBED = 256
HIDDEN = 64
HEADS = 4
NG = 64                      # graphs
EPS = 1e-5
NEG = 0.2
DW = 128                     # dst window
NT = 49                      # node tiles per core
PADN = NT * 128              # 6272 padded local rows
ASPL = 3072                  # local rows < ASPL go to table A
BROW = PADN - ASPL           # 3200 rows per core in table B
NA = NCORES * ASPL           # 24576 global A rows
NB = NCORES * BROW           # 25600 global B rows
ATILES = ASPL // 128         # 24
CH = int(os.environ.get("GAT_CHUNK", "24"))
GCALL = int(os.environ.get("GAT_GCALL", "8"))  # max blocks per dma_gather call
GAT_NOACC = os.environ.get("GAT_NOACC", "0") == "1"
GAT_AGLATE = os.environ.get("GAT_AGLATE", "0") == "1"
SUB = 4                      # ALD expansion subchunk (1 PSUM bank)
TRIG_A_W = 29                # window whose finalize emits next layer's AG-A

GAT_LAYERS = int(os.environ.get("GAT_LAYERS", "3"))

F32 = mybir.dt.float32
BF16 = mybir.dt.bfloat16
I16 = mybir.dt.int16
BF = ml_dtypes.bfloat16

TROW = 384                   # table row: 256 h bf16 + als as raw f32 + pad
JH = 264                     # h_ext width: 256 h + 4 al_s + 4 al_d

# consts tile column layout (f32)
C_BIAS = [0, 256, 512]
C_G = [768, 1024, 1280]
C_BE = [1536, 1792, 2048]
C_FCB = 2304
C_INV = 2560
C_EPS = 2624
C_R256 = 2625
C_IOTAC = 2626
C_SEL = 2688
CW = 3712


def _interleave_perm():
    p = np.zeros(256, np.int64)
    for h in range(HEADS):
        for c in range(HIDDEN):
            p[4 * c + h] = 64 * h + c
    return p


def _prep_weights(ws):
    """Fold attention vectors into W, apply channel interleave permutations."""
    perm = _interleave_perm()
    out = {}
    in_perm = np.arange(256)
    for l in range(3):
        W = np.asarray(ws[f"w{l}"], np.float64)
        a_s = np.asarray(ws[f"as{l}"], np.float64)
        a_d = np.asarray(ws[f"ad{l}"], np.float64)
        heads = HEADS if l < 2 else 1
        outc = HIDDEN if l < 2 else EMBED
        Wr = W.reshape(256, heads, outc)
        wa_s = np.einsum("khc,hc->kh", Wr, a_s)
        wa_d = np.einsum("khc,hc->kh", Wr, a_d)
        Wc = W.copy()
        out_perm = np.arange(256)
        if l < 2:
            Wc = Wc[:, perm]
            out_perm = perm
        Wx = np.zeros((256, JH), np.float64)
        Wx[:, :256] = Wc
        Wx[:, 256 : 256 + heads] = wa_s
        Wx[:, 260 : 260 + heads] = wa_d
        Wx = Wx[in_perm, :]
        out[f"wext{l}"] = Wx.astype(BF)
        out[f"bias{l}"] = np.asarray(ws[f"b{l}"], np.float64)[out_perm].astype(np.float32)
        out[f"g{l}"] = np.asarray(ws[f"g{l}"], np.float64)[out_perm].astype(np.float32)
        out[f"be{l}"] = np.asarray(ws[f"be{l}"], np.float64)[out_perm].astype(np.float32)
        in_perm = out_perm
    out["fc_w"] = np.asarray(ws["fc_w"], np.float32)
    out["fc_b"] = np.asarray(ws["fc_b"], np.float32)
    return out


def _prep_edges(edge_index, batch):
    """Partition/sort/pad edge structure. Self-loops are NOT added (handled
    analytically on device). Returns shared block structure + per-core data."""
    src = np.asarray(edge_index[0], np.int64)
    dst = np.asarray(edge_index[1], np.int64)
    # Random (i,i) edges stay in the stream; only the reference's appended
    # self-loop per node is handled analytically (slh fold) on device.
    lr = src % PARTN
    piece = (lr >= ASPL).astype(np.int64)
    srow = np.where(piece == 0, (src // PARTN) * ASPL + lr,
                    (src // PARTN) * BROW + (lr - ASPL))
    core = dst // PARTN

    per_core = []
    nblk = np.zeros((NCORES, NT, 2), np.int64)
    for c in range(NCORES):
        m = core == c
        s, d, h = srow[m], dst[m] - c * PARTN, piece[m]
        w = d // DW
        order = np.lexsort((s, h, w))
        s, d, w, h = s[order], d[order], w[order], h[order]
        per_core.append((s, d, w, h))
        cnts = np.bincount(w * 2 + h, minlength=NT * 2).reshape(NT, 2)
        nblk[c] = (cnts + 127) // 128

    NBLH = nblk.max(axis=0)  # [NT, 2]

    blocks = []  # (window, piece, first_of_window, last_of_window)
    blk_start = np.zeros((NT, 2), np.int64)
    pos = 0
    for wi in range(NT):
        tot = int(NBLH[wi, 0] + NBLH[wi, 1])
        blk_start[wi, 0] = pos
        for b in range(int(NBLH[wi, 0])):
            blocks.append((wi, 0, b == 0, b + 1 == tot))
        pos += int(NBLH[wi, 0])
        blk_start[wi, 1] = pos
        for b in range(int(NBLH[wi, 1])):
            blocks.append(
                (wi, 1, NBLH[wi, 0] == 0 and b == 0, b + 1 + NBLH[wi, 0] == tot)
            )
        pos += int(NBLH[wi, 1])
    NBLK = len(blocks)

    sidx = np.zeros((NCORES, 128, NBLK * 8), np.int16)
    dloc = np.full((NCORES, 128, NBLK), -1.0, np.float32)

    for c in range(NCORES):
        s, d, w, h = per_core[c]
        n = len(s)
        g = w * 2 + h
        starts = np.r_[0, np.flatnonzero(np.diff(g)) + 1]
        gstart = starts[np.searchsorted(g[starts], g)]
        pos_in = np.arange(n) - gstart
        blk = blk_start[w, h] + pos_in // 128
        p = pos_in % 128
        rows = p % 16
        cols = blk * 8 + p // 16
        sc = np.zeros((128, NBLK * 8), np.int16)
        sc[rows, cols] = s
        for k in range(1, 8):
            sc[16 * k : 16 * k + 16] = sc[:16]
        sidx[c] = sc
        dl = np.full((128, NBLK), -1.0, np.float32)
        dl[p, blk] = (d - w * DW).astype(np.float32)
        dloc[c] = dl

    b64 = np.asarray(batch, np.int64)
    nl = np.arange(128)[:, None] + 128 * np.arange(NT)[None, :]
    gid = np.zeros((NCORES, 128, NT), np.float32)
    for c in range(NCORES):
        valid = nl < PARTN
        gv = b64[c * PARTN + np.minimum(nl, PARTN - 1)]
        gid[c] = np.where(valid, gv.astype(np.float32), -1.0)
    cnt = np.bincount(b64, minlength=NG).astype(np.float64)
    inv_cnt = (1.0 / np.maximum(cnt, 1.0)).astype(np.float32)

    return blocks, NBLK, sidx, dloc, gid, inv_cnt


_PROGRAM_CACHE = {}
_LAST_RESULT = None


NQUEUES = int(os.environ.get("GAT_NQ", "4"))


def _build_program(blocks, NBLK):
    nc = bacc.Bacc(
        "TRN2",
        target_bir_lowering=False,
        debug=False,
        num_devices=NCORES,
        num_swdge_queues=NQUEUES,
    )
    mm = mybir.AluOpType
    AF = mybir.ActivationFunctionType

    # ---- inputs ----
    xT0 = nc.dram_tensor("xT0", [128, 2, PADN], BF16, kind="ExternalInput").ap()
    wext = [
        nc.dram_tensor(f"wext{l}", [256, JH], BF16, kind="ExternalInput").ap()
        for l in range(3)
    ]
    fcw = nc.dram_tensor("fcw", [256, 256], F32, kind="ExternalInput").ap()
    sidx_d = nc.dram_tensor("sidx", [128, NBLK * 8], I16, kind="ExternalInput").ap()
    dloc_d = nc.dram_tensor("dloc", [128, NBLK], F32, kind="ExternalInput").ap()
    gid_d = nc.dram_tensor("gid", [128, NT], F32, kind="ExternalInput").ap()
    iota_d = nc.dram_tensor("iota", [128, 128], F32, kind="ExternalInput").ap()
    ident_d = nc.dram_tensor("ident", [128, 128], F32, kind="ExternalInput").ap()
    consts_d = nc.dram_tensor("consts", [128, CW], F32, kind="ExternalInput").ap()
    out_t = nc.dram_tensor("out", [NG, 256], F32, kind="ExternalOutput").ap()

    # chunk plan
    chunks = []
    c0 = 0
    while c0 < NBLK:
        cb = min(CH, NBLK - c0)
        chunks.append((c0, cb))
        c0 += cb

    import contextlib

    with tile.TileContext(nc) as tc, contextlib.ExitStack() as _ctx:
        cpool = _ctx.enter_context(tc.tile_pool(name="const", bufs=1))
        spool = _ctx.enter_context(tc.tile_pool(name="struct", bufs=1))
        wpool = _ctx.enter_context(tc.tile_pool(name="w", bufs=1))
        slpool = _ctx.enter_context(tc.tile_pool(name="slh", bufs=2))
        gpool = _ctx.enter_context(tc.tile_pool(name="gath", bufs=2))
        cmpool = _ctx.enter_context(tc.tile_pool(name="cmp", bufs=2))
        npool = _ctx.enter_context(tc.tile_pool(name="nar", bufs=2))
        epool = _ctx.enter_context(tc.tile_pool(name="evac", bufs=2))
        ypool = _ctx.enter_context(tc.tile_pool(name="y", bufs=2))
        xpool = _ctx.enter_context(tc.tile_pool(name="xtw", bufs=2))
        smpool = _ctx.enter_context(tc.tile_pool(name="small", bufs=2))
        pct = _ctx.enter_context(tc.tile_pool(name="pct", bufs=1, space="PSUM"))
        prag = _ctx.enter_context(tc.tile_pool(name="prag", bufs=2, space="PSUM"))
        pald = _ctx.enter_context(tc.tile_pool(name="pald", bufs=1, space="PSUM"))
        pdt = _ctx.enter_context(tc.tile_pool(name="pdt", bufs=1, space="PSUM"))
        pfin = _ctx.enter_context(tc.tile_pool(name="pfin", bufs=2, space="PSUM"))
        dram = _ctx.enter_context(tc.tile_pool(name="dram", bufs=1, space="DRAM"))
        nc.gpsimd.load_library(library_config.mlp)

        # persistent SBUF
        iota_sb = cpool.tile([128, 128], F32, tag="iota")
        ident_sb = cpool.tile([128, 128], F32, tag="ident")
        consts_sb = cpool.tile([128, CW], F32, tag="consts")
        nc.sync.dma_start(iota_sb[:], iota_d[:])
        nc.sync.dma_start(ident_sb[:], ident_d[:])
        nc.sync.dma_start(consts_sb[:], consts_d[:])
        iota_bf = cpool.tile([128, 128], BF16, tag="iotabf")
        ident_bf = cpool.tile([128, 128], BF16, tag="identbf")
        nc.vector.tensor_copy(iota_bf[:], iota_sb[:])
        nc.vector.tensor_copy(ident_bf[:], ident_sb[:])
        dloc_f = spool.tile([128, NBLK], F32, tag="dlocf")
        dloc_bf = spool.tile([128, NBLK], BF16, tag="dlocbf")
        sidx_sb = spool.tile([128, NBLK * 8], I16, tag="sidx")
        gid_sb = spool.tile([128, NT], F32, tag="gid")
        nc.sync.dma_start(dloc_f[:], dloc_d[:])
        nc.sync.dma_start(sidx_sb[:], sidx_d[:])
        nc.sync.dma_start(gid_sb[:], gid_d[:])
        nc.vector.tensor_copy(dloc_bf[:], dloc_f[:])

        wsb = []
        for l in range(3):
            wt = wpool.tile([128, 2, JH], BF16, tag=f"w{l}")
            nc.sync.dma_start(wt[:], wext[l].rearrange("(k p) j -> p k j", p=128))
            wsb.append(wt)

        # per-layer DRAM tables
        tsA, tsB, tabA, tabB = [], [], [], []
        for l in range(GAT_LAYERS):
            tsA.append(dram.tile([ASPL, TROW], BF16, name=f"tsA{l}"))
            tsB.append(dram.tile([BROW, TROW], BF16, name=f"tsB{l}"))
            tabA.append(dram.tile([NA, TROW], BF16, addr_space="Shared", name=f"tabA{l}"))
            tabB.append(dram.tile([NB, TROW], BF16, addr_space="Shared", name=f"tabB{l}"))
        ar_in = dram.tile([NG, 256], F32)
        ar_out = dram.tile([NG, 256], F32, addr_space="Shared")

        def ag(l, which):
            src = tsA[l] if which == 0 else tsB[l]
            dst = tabA[l] if which == 0 else tabB[l]
            nc.gpsimd.collective_compute(
                "AllGather",
                mm.bypass,
                replica_groups=[list(range(NCORES))],
                ins=[src.opt()],
                outs=[dst.opt()],
            )

        # slh / alw ping-pong (producer: layer l-1 loop; consumer: layer l)
        def slh_tile():
            return slpool.tile([128, NT, 260], BF16, tag="slh", name="slh")

        def alw_tile():
            return slpool.tile([128, NT, 4], BF16, tag="alw", name="alw")

        pooled_ps = {}

        def emit_h(l_next, w, xtw, slh_nx, alw_nx):
            """Compute h^{l_next} for window w from transposed x (xtw),
            stash slh/alw, write the table row tile."""
            nh2 = HEADS if l_next < 2 else 1
            fin = pfin.tile([128, JH], F32, tag="fin")
            ph = fin[:]
            for kc in range(2):
                nc.tensor.matmul(
                    ph,
                    xtw[:, kc, :],
                    wsb[l_next][:, kc, :],
                    start=(kc == 0),
                    stop=(kc == 1),
                )
            # self-loop factor: sl = exp(lrelu(als+ald))
            als8 = smpool.tile([128, 8], F32, tag="als8")
            nc.vector.tensor_copy(als8[:], ph[:, 256:264])
            sl = smpool.tile([128, 4], F32, tag="sl")
            nc.vector.tensor_tensor(
                sl[:, 0:nh2], als8[:, 0:nh2], als8[:, 4 : 4 + nh2], mm.add
            )
            slr = smpool.tile([128, 4], F32, tag="slr")
            nc.vector.scalar_tensor_tensor(
                slr[:, 0:nh2], sl[:, 0:nh2], NEG, sl[:, 0:nh2], mm.mult, mm.max
            )
            exf = smpool.tile([128, 4], F32, tag="exf")
            nc.scalar.activation(exf[:, 0:nh2], slr[:, 0:nh2], AF.Exp)
            nc.vector.tensor_copy(slh_nx[:, w, 256 : 256 + nh2], exf[:, 0:nh2])
            if nh2 == 4:
                slh_v = slh_nx[:, w, 0:256].rearrange("p (c h) -> p c h", h=4)
                ph_v = ph[:, 0:256].rearrange("p (c h) -> p c h", h=4)
                ex_b = exf[:].unsqueeze(1).broadcast_to([128, 64, 4])
                nc.vector.tensor_tensor(slh_v, ph_v, ex_b, mm.mult)
            else:
                ex_b = exf[:, 0:1].broadcast_to([128, 256])
                nc.vector.tensor_tensor(
                    slh_nx[:, w, 0:256], ph[:, 0:256], ex_b, mm.mult
                )
            nc.vector.tensor_copy(alw_nx[:, w, 0:nh2], ph[:, 260 : 260 + nh2])
            ev = epool.tile([128, TROW], BF16, tag="ev")
            nc.scalar.activation(ev[:, 0:256], ph[:, 0:256], AF.Copy)
            nc.vector.tensor_copy(ev[:, 256:264].bitcast(F32), ph[:, 256:260])
            nc.vector.memset(ev[:, 264:TROW], 0.0)
            if w < ATILES:
                dst = tsA[l_next][:].rearrange("(t p) r -> p t r", p=128)[:, w, :]
            else:
                dst = tsB[l_next][:].rearrange("(t p) r -> p t r", p=128)[:, w - ATILES, :]
            nc.sync.dma_start(dst, ev[:])

        def finalize_window(l, w, pw, slh_nx, alw_nx):
            nh = HEADS if l < 2 else 1
            # den > 0 always: the analytic self-loop term contributes exp(..)
            rden = smpool.tile([128, 4], F32, tag="rden")
            nc.vector.reciprocal(rden[:, 0:nh], pw[:, 256 : 256 + nh])
            y = ypool.tile([128, 256], F32, tag="y")
            if l < 2:
                y_v = y[:].rearrange("p (c h) -> p c h", h=4)
                p_v = pw[:, 0:256].rearrange("p (c h) -> p c h", h=4)
                rd_b = rden[:].unsqueeze(1).broadcast_to([128, 64, 4])
                nc.vector.tensor_tensor(y_v, p_v, rd_b, mm.mult)
            else:
                rd_b = rden[:, 0:1].broadcast_to([128, 256])
                nc.vector.tensor_tensor(y[:], pw[:, 0:256], rd_b, mm.mult)
            bias_b = consts_sb[:, C_BIAS[l] : C_BIAS[l] + 256]
            # bias add fused with the LN mean accumulation
            s1 = smpool.tile([128, 1], F32, tag="s1")
            nc.vector.scalar_tensor_tensor(
                y[:], y[:], 0.0, bias_b, mm.add, mm.add, accum_out=s1[:]
            )
            ysq = ypool.tile([128, 256], F32, tag="ysq")
            ss = smpool.tile([128, 1], F32, tag="ss")
            if GAT_NOACC:
                nc.vector.tensor_tensor(ysq[:], y[:], y[:], mm.mult)
                nc.vector.tensor_reduce(ss[:], ysq[:], mybir.AxisListType.X, mm.add)
            else:
                nc.scalar.activation(ysq[:], y[:], AF.Square, accum_out=ss[:])
            mu = smpool.tile([128, 1], F32, tag="mu")
            nc.vector.tensor_scalar_mul(mu[:], s1[:], 1.0 / 256.0)
            mu2 = smpool.tile([128, 1], F32, tag="mu2")
            nc.vector.tensor_tensor(mu2[:], mu[:], mu[:], mm.mult)
            var = smpool.tile([128, 1], F32, tag="var")
            nc.vector.scalar_tensor_tensor(
                var[:], ss[:], 1.0 / 256.0, mu2[:], mm.mult, mm.subtract
            )
            sd = smpool.tile([128, 1], F32, tag="sd")
            nc.scalar.activation(
                sd[:], var[:], AF.Sqrt, bias=consts_sb[:, C_EPS : C_EPS + 1]
            )
            rstd = smpool.tile([128, 1], F32, tag="rstd")
            nc.vector.reciprocal(rstd[:], sd[:])
            nmr = smpool.tile([128, 1], F32, tag="nmr")
            nc.vector.scalar_tensor_tensor(
                nmr[:], mu[:], -1.0, rstd[:], mm.mult, mm.mult
            )
            # y1 = y*rstd - mu*rstd
            nc.scalar.activation(
                y[:], y[:], AF.Identity, bias=nmr[:], scale=rstd[:]
            )
            nc.vector.tensor_tensor(
                y[:], y[:], consts_sb[:, C_G[l] : C_G[l] + 256], mm.mult
            )
            nc.vector.tensor_tensor(
                y[:], y[:], consts_sb[:, C_BE[l] : C_BE[l] + 256], mm.add
            )
            # ELU: max(y,0) + min(exp(y),1) - 1
            ee = ypool.tile([128, 256], F32, tag="ee")
            nc.scalar.activation(ee[:], y[:], AF.Exp)
            nc.vector.tensor_scalar_min(ee[:], ee[:], 1.0)
            nc.vector.tensor_scalar_max(y[:], y[:], 0.0)
            nc.vector.scalar_tensor_tensor(y[:], y[:], -1.0, ee[:], mm.add, mm.add)

            if l + 1 < GAT_LAYERS:
                xtw = xpool.tile([128, 2, 128], BF16, tag="xtw")
                for kc in range(2):
                    pt = pdt.tile([128, 128], F32, tag="pt")
                    nc.tensor.transpose(pt[:], y[:, kc * 128 : (kc + 1) * 128], ident_sb[:])
                    nc.scalar.activation(xtw[:, kc, :], pt[:], AF.Copy)
                emit_h(l + 1, w, xtw, slh_nx, alw_nx)
                if w == TRIG_A_W and not GAT_AGLATE:
                    ag(l + 1, 0)
            if l == GAT_LAYERS - 1:
                cmbf = smpool.tile([128, NG], BF16, tag="cmbf")
                gid_b = gid_sb[:, w : w + 1].broadcast_to([128, NG])
                nc.vector.tensor_tensor(cmbf[:], gid_b, iota_sb[:, 0:NG], mm.is_equal)
                ybf = ypool.tile([128, 256], BF16, tag="ybf")
                nc.scalar.activation(ybf[:], y[:], AF.Copy)
                pp = pooled_ps["pp"]
                nc.tensor.matmul(
                    pp[:],
                    cmbf[:],
                    ybf[:],
                    start=(w == 0),
                    stop=(w == NT - 1),
                    skip_group_check=True,
                )

        # ---- prologue: table(0) from input x ----
        slh_cur = slh_tile()
        alw_cur = alw_tile()
        for w in range(NT):
            xtw = xpool.tile([128, 2, 128], BF16, tag="xtw")
            nc.sync.dma_start(xtw[:], xT0[:, :, w * 128 : (w + 1) * 128])
            emit_h(0, w, xtw, slh_cur, alw_cur)
            if w == ATILES - 1 and not GAT_AGLATE:
                ag(0, 0)
        if GAT_AGLATE:
            ag(0, 0)
        ag(0, 1)

        # ---- layer loops ----
        for l in range(GAT_LAYERS):
            nh = HEADS if l < 2 else 1
            if l + 1 < GAT_LAYERS:
                slh_nx = slh_tile()
                alw_nx = alw_tile()
            else:
                slh_nx = alw_nx = None
            if l == GAT_LAYERS - 1:
                pooled_ps["pp"] = prag.tile(
                    [NG, 256], F32, tag="pp", bufs=1, name="pp"
                )
            win_psum = {}
            gq = [0]
            for ci, (c0, cb) in enumerate(chunks):
                G = gpool.tile([128, CH, TROW], BF16, tag="G")
                # gather calls per piece-run, round-robined over SWDGE queues
                r0 = 0
                while r0 < cb:
                    pc = blocks[c0 + r0][1]
                    r1 = r0
                    while (
                        r1 < cb
                        and blocks[c0 + r1][1] == pc
                        and r1 - r0 < GCALL
                    ):
                        r1 += 1
                    nrun = (r1 - r0) * 128
                    nc.gpsimd.dma_gather(
                        G[:, r0:r1, :],
                        (tabA[l] if pc == 0 else tabB[l])[:],
                        sidx_sb[:, (c0 + r0) * 8 : (c0 + r1) * 8],
                        nrun,
                        nrun,
                        TROW,
                        queue_num=gq[0] % NQUEUES,
                    )
                    gq[0] += 1
                    r0 = r1
                # cmp masks (batched)
                cmp = cmpool.tile([128, CH, DW], BF16, tag="cmp")
                dl_b = (
                    dloc_bf[:, c0 : c0 + cb].unsqueeze(2).broadcast_to([128, cb, DW])
                )
                io_b = iota_bf[:].unsqueeze(1).broadcast_to([128, cb, DW])
                nc.vector.tensor_tensor(cmp[:, 0:cb, :], io_b, dl_b, mm.is_equal)
                # ALD expansion: cmpT via PE transpose of cmp, then matmul vs alw
                ald_ps = pald.tile([128, 4 * CH], F32, tag="ald")
                for s0 in range(0, cb, SUB):
                    sbk = min(SUB, cb - s0)
                    ct_ps = pct.tile([128, SUB, 128], BF16, tag="ct")
                    for j in range(sbk):
                        nc.tensor.matmul(
                            ct_ps[:, j, :],
                            cmp[:, s0 + j, :],
                            ident_bf[:],
                            is_transpose=True,
                            start=(j == 0),
                            stop=(j == sbk - 1),
                            skip_group_check=True,
                        )
                    cmpt = npool.tile([128, SUB, 128], BF16, tag="cmpt")
                    nc.vector.tensor_copy(cmpt[:, 0:sbk, :], ct_ps[:, 0:sbk, :])
                    for j in range(sbk):
                        b = s0 + j
                        nc.tensor.matmul(
                            ald_ps[:, nh * b : nh * b + nh],
                            cmpt[:, j, :],
                            alw_cur[:, blocks[c0 + b][0], 0:nh],
                            start=(b == 0),
                            stop=(b == cb - 1),
                            skip_group_check=True,
                        )
                # attention pointwise (batched per chunk)
                nar = npool.tile([128, CH * 4], F32, tag="nar")
                nar_v = nar[:, 0 : nh * cb].rearrange("p (b a) -> p b a", a=nh)
                als_v = G[:, 0:cb, 256:264].bitcast(F32)[:, :, 0:nh]
                ald_v = ald_ps[:, 0 : nh * cb].rearrange("p (b a) -> p b a", a=nh)
                nc.vector.tensor_tensor(nar_v, als_v, ald_v, mm.add)
                lrt = npool.tile([128, CH * 4], F32, tag="lrt")
                nc.vector.scalar_tensor_tensor(
                    lrt[:, 0 : nh * cb], nar[:, 0 : nh * cb], NEG,
                    nar[:, 0 : nh * cb], mm.mult, mm.max,
                )
                lrt_v = lrt[:, 0 : nh * cb].rearrange("p (b a) -> p b a", a=nh)
                nc.scalar.activation(
                    G[:, 0:cb, 256 : 256 + nh], lrt_v, AF.Exp
                )
                if nh == 1:
                    # replicate ex to cols 257:260 so prescale uses the fast
                    # 4-wide broadcast pattern
                    nc.vector.tensor_copy(
                        G[:, 0:cb, 257:260],
                        G[:, 0:cb, 256:257].broadcast_to([128, cb, 3]),
                    )
                # prescale h by ex
                g_v = G[:, 0:cb, 0:256].rearrange("p b (c h) -> p b c h", h=4)
                ex_b = (
                    G[:, 0:cb, 256:260].unsqueeze(2).broadcast_to([128, cb, 64, 4])
                )
                nc.vector.tensor_tensor(g_v, g_v, ex_b, mm.mult)
                # aggregation
                for b in range(cb):
                    wi, pc, first, last = blocks[c0 + b]
                    if first:
                        pw = prag.tile([128, 260], F32, tag="agg")
                        win_psum[wi] = pw
                        nc.tensor.matmul(
                            pw[:, 0 : 256 + nh],
                            ident_bf[:],
                            slh_cur[:, wi, 0 : 256 + nh],
                            start=True,
                            stop=False,
                            skip_group_check=True,
                        )
                    pw = win_psum[wi]
                    nc.tensor.matmul(
                        pw[:, 0 : 256 + nh],
                        cmp[:, b, :],
                        G[:, b, 0 : 256 + nh],
                        start=False,
                        stop=last,
                        skip_group_check=True,
                    )
                    if last:
                        finalize_window(l, wi, pw, slh_nx, alw_nx)
                        del win_psum[wi]
            if l + 1 < GAT_LAYERS:
                if GAT_AGLATE:
                    ag(l + 1, 0)
                ag(l + 1, 1)
            slh_cur, alw_cur = slh_nx, alw_nx

        # ---- tail: AllReduce pooled sums + FC ----
        pooled = smpool.tile([NG, 256], F32, tag="pooled")
        nc.vector.tensor_copy(pooled[:], pooled_ps["pp"][:])
        nc.sync.dma_start(ar_in[:], pooled[:])
        nc.gpsimd.collective_compute(
            "AllReduce",
            mm.add,
            replica_groups=[list(range(NCORES))],
            ins=[ar_in.opt()],
            outs=[ar_out.opt()],
        )
        pooled2 = smpool.tile([NG, 256], F32, tag="pooled2")
        nc.sync.dma_start(pooled2[:], ar_out[:])
        nc.vector.tensor_scalar_mul(
            pooled2[:], pooled2[:], consts_sb[0:NG, C_INV : C_INV + 1]
        )
        fcw_sb = wpool.tile([128, 2, 256], F32, tag="fcw")
        nc.sync.dma_start(fcw_sb[:], fcw.rearrange("(k p) j -> p k j", p=128))
        poolT = smpool.tile([128, 2, NG], F32, tag="poolT")
        for kc in range(2):
            pt2 = pdt.tile([128, 128], F32, tag="pt")
            nc.tensor.transpose(
                pt2[0:128, 0:NG],
                pooled2[:, kc * 128 : (kc + 1) * 128],
                ident_sb[0:NG, 0:NG],
            )
            nc.vector.tensor_copy(poolT[:, kc, :], pt2[0:128, 0:NG])
        pfc = prag.tile([NG, 256], F32, tag="pp", bufs=1)
        for kc in range(2):
            nc.tensor.matmul(
                pfc[:],
                poolT[:, kc, :],
                fcw_sb[:, kc, :],
                start=(kc == 0),
                stop=(kc == 1),
            )
        ores = smpool.tile([NG, 256], F32, tag="ores")
        fcb_b = consts_sb[0:NG, C_FCB : C_FCB + 256]
        nc.vector.tensor_tensor(ores[:], pfc[:], fcb_b, mm.add)
        nc.vector.tensor_scalar_max(ores[:], ores[:], 0.0)
        nc.sync.dma_start(out_t[:], ores[:])

    nc.compile()
    return nc


def kernel(**inputs):
    x = np.asarray(inputs["x"], np.float32)
    edge_index = np.asarray(inputs["edge_index"])
    batch = np.asarray(inputs["batch"])

    blocks, NBLK, sidx, dloc, gid, inv_cnt = _prep_edges(edge_index, batch)
    wp = _prep_weights(inputs)

    key = (NBLK, CH, GAT_LAYERS, tuple(blocks[:8]))
    if key not in _PROGRAM_CACHE:
        _PROGRAM_CACHE[key] = _build_program(blocks, NBLK)
    nc = _PROGRAM_CACHE[key]

    iota = np.broadcast_to(np.arange(128, dtype=np.float32), (128, 128)).copy()
    ident = np.eye(128, dtype=np.float32)
    consts = np.zeros((128, CW), np.float32)
    for l in range(3):
        consts[:, C_BIAS[l] : C_BIAS[l] + 256] = wp[f"bias{l}"][None, :]
        consts[:, C_G[l] : C_G[l] + 256] = wp[f"g{l}"][None, :]
        consts[:, C_BE[l] : C_BE[l] + 256] = wp[f"be{l}"][None, :]
    consts[:, C_FCB : C_FCB + 256] = wp["fc_b"][None, :]
    consts[:NG, C_INV] = inv_cnt
    consts[NG:, C_INV] = 1.0
    consts[:, C_EPS] = EPS
    consts[:, C_R256] = 1.0 / 256.0
    consts[:, C_IOTAC] = np.arange(128, dtype=np.float32)
    for q in range(8):
        consts[q, C_SEL + 128 * q : C_SEL + 128 * (q + 1)] = 1.0

    in_maps = []
    for c in range(NCORES):
        xs = np.zeros((PADN, 256), np.float32)
        xs[:PARTN] = x[c * PARTN : (c + 1) * PARTN]
        xT0 = np.ascontiguousarray(
            xs.T.reshape(2, 128, PADN).transpose(1, 0, 2)
        ).astype(BF)
        in_maps.append(
            {
                "xT0": xT0,
                "wext0": wp["wext0"],
                "wext1": wp["wext1"],
                "wext2": wp["wext2"],
                "fcw": wp["fc_w"],
                "sidx": sidx[c],
                "dloc": dloc[c],
                "gid": gid[c],
                "iota": iota,
                "ident": ident,
                "consts": consts,
            }
        )

    global _LAST_RESULT
    res = run_bass_kernel_spmd(nc, in_maps, core_ids=list(range(NCORES)), trace=False)
    _LAST_RESULT = res
    return res.results[0]["out"]


# revision 46
# speedup vs baseline: 1.9712x; 1.0156x over previous
"""3-layer GAT + global-mean-pool + FC on 8 Trainium2 NeuronCores.

Graph/data-parallel per the sharding hint: nodes and their incident (dst)
edges are sharded across 8 cores; weights are replicated.

v2 pipeline (vs. the phase-serial baseline):
  - Per 128-dst window, aggregation finalize immediately runs bias+LN+ELU,
    transposes the result, computes the NEXT layer's h tile and writes it to
    the next layer's table slice. No batched LN phase, no separate
    transpose/phase-1 passes.
  - The per-layer table is split in two row ranges (A: local rows <3072,
    B: rest). Each half is AllGathered separately as soon as its tiles are
    written, so the collectives hide under gather processing of the previous
    half/layer. The A/B split also keeps gather indices within int16.
  - Self-loop edges are removed from the gather stream entirely: their
    contribution exp(lrelu(als+ald))*h (plus the denominator term) is
    precomputed per node at h-time (slh) and folded into the window PSUM
    with one identity matmul.
  - One fused matmul per 128-edge block: the exp'd logits are written into
    the gathered rows' spare columns so messages and softmax denominators
    accumulate in a single [128x128]x[128x260] matmul.
  - bf16 everywhere on the PE (x, W, tables, cmp masks); attention
    pointwise math on ACT (Lrelu/Exp) and batched DVE ops per chunk.
"""

import os
import sys

for _p in ("/opt/trn_rl_repo", "/opt/pypackages"):
    if _p not in sys.path:
        sys.path.append(_p)

import numpy as np
import ml_dtypes

import concourse.bass as bass
import concourse.bacc as bacc
import concourse.tile as tile
import concourse.mybir as mybir
from concourse import library_config
from concourse.bass_utils import run_bass_kernel_spmd

# ---- problem constants (hardcoded per spec) ----
N = 50000
NCORES = 8
PARTN = N // NCORES          # 6250 real nodes per core
EMBED = 256
HIDDEN = 64
HEADS = 4
NG = 64                      # graphs
EPS = 1e-5
NEG = 0.2
DW = 128                     # dst window
NT = 49                      # node tiles per core
PADN = NT * 128              # 6272 padded local rows
ASPL = 3072                  # local rows < ASPL go to table A
BROW = PADN - ASPL           # 3200 rows per core in table B
NA = NCORES * ASPL           # 24576 global A rows
NB = NCORES * BROW           # 25600 global B rows
ATILES = ASPL // 128         # 24
CH = int(os.environ.get("GAT_CHUNK", "24"))
GCALL = int(os.environ.get("GAT_GCALL", "8"))  # max blocks per dma_gather call
GAT_NOACC = os.environ.get("GAT_NOACC", "0") == "1"
GAT_AGLATE = os.environ.get("GAT_AGLATE", "0") == "1"
SUB = 4                      # ALD expansion subchunk (1 PSUM bank)
TRIG_A_W = 29                # window whose finalize emits next layer's AG-A

GAT_LAYERS = int(os.environ.get("GAT_LAYERS", "3"))

F32 = mybir.dt.float32
BF16 = mybir.dt.bfloat16
I16 = mybir.dt.int16
BF = ml_dtypes.bfloat16

TROW = 384                   # table row: 256 h bf16 + als as raw f32 + pad
JH = 264                     # h_ext width: 256 h + 4 al_s + 4 al_d

# consts tile column layout (f32)
C_BIAS = [0, 256, 512]
C_G = [768, 1024, 1280]
C_BE = [1536, 1792, 2048]
C_FCB = 2304
C_INV = 2560
C_EPS = 2624
C_R256 = 2625
C_IOTAC = 2626
C_SEL = 2688
CW = 3712


def _interleave_perm():
    p = np.zeros(256, np.int64)
    for h in range(HEADS):
        for c in range(HIDDEN):
            p[4 * c + h] = 64 * h + c
    return p


def _prep_weights(ws):
    """Fold attention vectors into W, apply channel interleave permutations."""
    perm = _interleave_perm()
    out = {}
    in_perm = np.arange(256)
    for l in range(3):
        W = np.asarray(ws[f"w{l}"], np.float64)
        a_s = np.asarray(ws[f"as{l}"], np.float64)
        a_d = np.asarray(ws[f"ad{l}"], np.float64)
        heads = HEADS if l < 2 else 1
        outc = HIDDEN if l < 2 else EMBED
        Wr = W.reshape(256, heads, outc)
        wa_s = np.einsum("khc,hc->kh", Wr, a_s)
        wa_d = np.einsum("khc,hc->kh", Wr, a_d)
        Wc = W.copy()
        out_perm = np.arange(256)
        if l < 2:
            Wc = Wc[:, perm]
            out_perm = perm
        Wx = np.zeros((256, JH), np.float64)
        Wx[:, :256] = Wc
        Wx[:, 256 : 256 + heads] = wa_s
        Wx[:, 260 : 260 + heads] = wa_d
        Wx = Wx[in_perm, :]
        out[f"wext{l}"] = Wx.astype(BF)
        out[f"bias{l}"] = np.asarray(ws[f"b{l}"], np.float64)[out_perm].astype(np.float32)
        out[f"g{l}"] = np.asarray(ws[f"g{l}"], np.float64)[out_perm].astype(np.float32)
        out[f"be{l}"] = np.asarray(ws[f"be{l}"], np.float64)[out_perm].astype(np.float32)
        in_perm = out_perm
    out["fc_w"] = np.asarray(ws["fc_w"], np.float32)
    out["fc_b"] = np.asarray(ws["fc_b"], np.float32)
    return out


def _prep_edges(edge_index, batch):
    """Partition/sort/pad edge structure. Self-loops are NOT added (handled
    analytically on device). Returns shared block structure + per-core data."""
    src = np.asarray(edge_index[0], np.int64)
    dst = np.asarray(edge_index[1], np.int64)
    # Random (i,i) edges stay in the stream; only the reference's appended
    # self-loop per node is handled analytically (slh fold) on device.
    lr = src % PARTN
    piece = (lr >= ASPL).astype(np.int64)
    srow = np.where(piece == 0, (src // PARTN) * ASPL + lr,
                    (src // PARTN) * BROW + (lr - ASPL))
    core = dst // PARTN

    per_core = []
    nblk = np.zeros((NCORES, NT, 2), np.int64)
    for c in range(NCORES):
        m = core == c
        s, d, h = srow[m], dst[m] - c * PARTN, piece[m]
        w = d // DW
        order = np.lexsort((s, h, w))
        s, d, w, h = s[order], d[order], w[order], h[order]
        per_core.append((s, d, w, h))
        cnts = np.bincount(w * 2 + h, minlength=NT * 2).reshape(NT, 2)
        nblk[c] = (cnts + 127) // 128

    NBLH = nblk.max(axis=0)  # [NT, 2]

    blocks = []  # (window, piece, first_of_window, last_of_window)
    blk_start = np.zeros((NT, 2), np.int64)
    pos = 0
    for wi in range(NT):
        tot = int(NBLH[wi, 0] + NBLH[wi, 1])
        blk_start[wi, 0] = pos
        for b in range(int(NBLH[wi, 0])):
            blocks.append((wi, 0, b == 0, b + 1 == tot))
        pos += int(NBLH[wi, 0])
        blk_start[wi, 1] = pos
        for b in range(int(NBLH[wi, 1])):
            blocks.append(
                (wi, 1, NBLH[wi, 0] == 0 and b == 0, b + 1 + NBLH[wi, 0] == tot)
            )
        pos += int(NBLH[wi, 1])
    NBLK = len(blocks)

    sidx = np.zeros((NCORES, 128, NBLK * 8), np.int16)
    dloc = np.full((NCORES, 128, NBLK), -1.0, np.float32)

    for c in range(NCORES):
        s, d, w, h = per_core[c]
        n = len(s)
        g = w * 2 + h
        starts = np.r_[0, np.flatnonzero(np.diff(g)) + 1]
        gstart = starts[np.searchsorted(g[starts], g)]
        pos_in = np.arange(n) - gstart
        blk = blk_start[w, h] + pos_in // 128
        p = pos_in % 128
        rows = p % 16
        cols = blk * 8 + p // 16
        sc = np.zeros((128, NBLK * 8), np.int16)
        sc[rows, cols] = s
        for k in range(1, 8):
            sc[16 * k : 16 * k + 16] = sc[:16]
        sidx[c] = sc
        dl = np.full((128, NBLK), -1.0, np.float32)
        dl[p, blk] = (d - w * DW).astype(np.float32)
        dloc[c] = dl

    b64 = np.asarray(batch, np.int64)
    nl = np.arange(128)[:, None] + 128 * np.arange(NT)[None, :]
    gid = np.zeros((NCORES, 128, NT), np.float32)
    for c in range(NCORES):
        valid = nl < PARTN
        gv = b64[c * PARTN + np.minimum(nl, PARTN - 1)]
        gid[c] = np.where(valid, gv.astype(np.float32), -1.0)
    cnt = np.bincount(b64, minlength=NG).astype(np.float64)
    inv_cnt = (1.0 / np.maximum(cnt, 1.0)).astype(np.float32)

    return blocks, NBLK, sidx, dloc, gid, inv_cnt


_PROGRAM_CACHE = {}
_LAST_RESULT = None


NQUEUES = int(os.environ.get("GAT_NQ", "4"))


def _build_program(blocks, NBLK):
    nc = bacc.Bacc(
        "TRN2",
        target_bir_lowering=False,
        debug=False,
        num_devices=NCORES,
        num_swdge_queues=NQUEUES,
    )
    mm = mybir.AluOpType
    AF = mybir.ActivationFunctionType

    # ---- inputs ----
    xT0 = nc.dram_tensor("xT0", [128, 2, PADN], BF16, kind="ExternalInput").ap()
    wext = [
        nc.dram_tensor(f"wext{l}", [256, JH], BF16, kind="ExternalInput").ap()
        for l in range(3)
    ]
    fcw = nc.dram_tensor("fcw", [256, 256], F32, kind="ExternalInput").ap()
    sidx_d = nc.dram_tensor("sidx", [128, NBLK * 8], I16, kind="ExternalInput").ap()
    dloc_d = nc.dram_tensor("dloc", [128, NBLK], F32, kind="ExternalInput").ap()
    gid_d = nc.dram_tensor("gid", [128, NT], F32, kind="ExternalInput").ap()
    iota_d = nc.dram_tensor("iota", [128, 128], F32, kind="ExternalInput").ap()
    ident_d = nc.dram_tensor("ident", [128, 128], F32, kind="ExternalInput").ap()
    consts_d = nc.dram_tensor("consts", [128, CW], F32, kind="ExternalInput").ap()
    out_t = nc.dram_tensor("out", [NG, 256], F32, kind="ExternalOutput").ap()

    # chunk plan
    chunks = []
    c0 = 0
    while c0 < NBLK:
        cb = min(CH, NBLK - c0)
        chunks.append((c0, cb))
        c0 += cb

    import contextlib

    with tile.TileContext(nc) as tc, contextlib.ExitStack() as _ctx:
        cpool = _ctx.enter_context(tc.tile_pool(name="const", bufs=1))
        spool = _ctx.enter_context(tc.tile_pool(name="struct", bufs=1))
        wpool = _ctx.enter_context(tc.tile_pool(name="w", bufs=1))
        slpool = _ctx.enter_context(tc.tile_pool(name="slh", bufs=2))
        gpool = _ctx.enter_context(tc.tile_pool(name="gath", bufs=3))
        cmpool = _ctx.enter_context(tc.tile_pool(name="cmp", bufs=2))
        npool = _ctx.enter_context(tc.tile_pool(name="nar", bufs=2))
        epool = _ctx.enter_context(tc.tile_pool(name="evac", bufs=2))
        ypool = _ctx.enter_context(tc.tile_pool(name="y", bufs=2))
        xpool = _ctx.enter_context(tc.tile_pool(name="xtw", bufs=2))
        smpool = _ctx.enter_context(tc.tile_pool(name="small", bufs=2))
        pct = _ctx.enter_context(tc.tile_pool(name="pct", bufs=1, space="PSUM"))
        prag = _ctx.enter_context(tc.tile_pool(name="prag", bufs=2, space="PSUM"))
        pald = _ctx.enter_context(tc.tile_pool(name="pald", bufs=1, space="PSUM"))
        pdt = _ctx.enter_context(tc.tile_pool(name="pdt", bufs=1, space="PSUM"))
        pfin = _ctx.enter_context(tc.tile_pool(name="pfin", bufs=2, space="PSUM"))
        dram = _ctx.enter_context(tc.tile_pool(name="dram", bufs=1, space="DRAM"))
        nc.gpsimd.load_library(library_config.mlp)

        # persistent SBUF
        iota_sb = cpool.tile([128, 128], F32, tag="iota")
        ident_sb = cpool.tile([128, 128], F32, tag="ident")
        consts_sb = cpool.tile([128, CW], F32, tag="consts")
        nc.sync.dma_start(iota_sb[:], iota_d[:])
        nc.sync.dma_start(ident_sb[:], ident_d[:])
        nc.sync.dma_start(consts_sb[:], consts_d[:])
        iota_bf = cpool.tile([128, 128], BF16, tag="iotabf")
        ident_bf = cpool.tile([128, 128], BF16, tag="identbf")
        nc.vector.tensor_copy(iota_bf[:], iota_sb[:])
        nc.vector.tensor_copy(ident_bf[:], ident_sb[:])
        dloc_f = spool.tile([128, NBLK], F32, tag="dlocf")
        dloc_bf = spool.tile([128, NBLK], BF16, tag="dlocbf")
        sidx_sb = spool.tile([128, NBLK * 8], I16, tag="sidx")
        gid_sb = spool.tile([128, NT], F32, tag="gid")
        nc.sync.dma_start(dloc_f[:], dloc_d[:])
        nc.sync.dma_start(sidx_sb[:], sidx_d[:])
        nc.sync.dma_start(gid_sb[:], gid_d[:])
        nc.vector.tensor_copy(dloc_bf[:], dloc_f[:])

        wsb = []
        for l in range(3):
            wt = wpool.tile([128, 2, JH], BF16, tag=f"w{l}")
            nc.sync.dma_start(wt[:], wext[l].rearrange("(k p) j -> p k j", p=128))
            wsb.append(wt)

        # per-layer DRAM tables
        tsA, tsB, tabA, tabB = [], [], [], []
        for l in range(GAT_LAYERS):
            tsA.append(dram.tile([ASPL, TROW], BF16, name=f"tsA{l}"))
            tsB.append(dram.tile([BROW, TROW], BF16, name=f"tsB{l}"))
            tabA.append(dram.tile([NA, TROW], BF16, addr_space="Shared", name=f"tabA{l}"))
            tabB.append(dram.tile([NB, TROW], BF16, addr_space="Shared", name=f"tabB{l}"))
        ar_in = dram.tile([NG, 256], F32)
        ar_out = dram.tile([NG, 256], F32, addr_space="Shared")

        def ag(l, which):
            src = tsA[l] if which == 0 else tsB[l]
            dst = tabA[l] if which == 0 else tabB[l]
            nc.gpsimd.collective_compute(
                "AllGather",
                mm.bypass,
                replica_groups=[list(range(NCORES))],
                ins=[src.opt()],
                outs=[dst.opt()],
            )

        # slh / alw ping-pong (producer: layer l-1 loop; consumer: layer l)
        def slh_tile():
            return slpool.tile([128, NT, 260], BF16, tag="slh", name="slh")

        def alw_tile():
            return slpool.tile([128, NT, 4], BF16, tag="alw", name="alw")

        pooled_ps = {}

        def emit_h(l_next, w, xtw, slh_nx, alw_nx):
            """Compute h^{l_next} for window w from transposed x (xtw),
            stash slh/alw, write the table row tile."""
            nh2 = HEADS if l_next < 2 else 1
            fin = pfin.tile([128, JH], F32, tag="fin")
            ph = fin[:]
            for kc in range(2):
                nc.tensor.matmul(
                    ph,
                    xtw[:, kc, :],
                    wsb[l_next][:, kc, :],
                    start=(kc == 0),
                    stop=(kc == 1),
                )
            # self-loop factor: sl = exp(lrelu(als+ald))
            als8 = smpool.tile([128, 8], F32, tag="als8")
            nc.vector.tensor_copy(als8[:], ph[:, 256:264])
            sl = smpool.tile([128, 4], F32, tag="sl")
            nc.vector.tensor_tensor(
                sl[:, 0:nh2], als8[:, 0:nh2], als8[:, 4 : 4 + nh2], mm.add
            )
            slr = smpool.tile([128, 4], F32, tag="slr")
            nc.vector.scalar_tensor_tensor(
                slr[:, 0:nh2], sl[:, 0:nh2], NEG, sl[:, 0:nh2], mm.mult, mm.max
            )
            exf = smpool.tile([128, 4], F32, tag="exf")
            nc.scalar.activation(exf[:, 0:nh2], slr[:, 0:nh2], AF.Exp)
            nc.vector.tensor_copy(slh_nx[:, w, 256 : 256 + nh2], exf[:, 0:nh2])
            if nh2 == 4:
                slh_v = slh_nx[:, w, 0:256].rearrange("p (c h) -> p c h", h=4)
                ph_v = ph[:, 0:256].rearrange("p (c h) -> p c h", h=4)
                ex_b = exf[:].unsqueeze(1).broadcast_to([128, 64, 4])
                nc.vector.tensor_tensor(slh_v, ph_v, ex_b, mm.mult)
            else:
                ex_b = exf[:, 0:1].broadcast_to([128, 256])
                nc.vector.tensor_tensor(
                    slh_nx[:, w, 0:256], ph[:, 0:256], ex_b, mm.mult
                )
            nc.vector.tensor_copy(alw_nx[:, w, 0:nh2], ph[:, 260 : 260 + nh2])
            ev = epool.tile([128, TROW], BF16, tag="ev")
            nc.scalar.activation(ev[:, 0:256], ph[:, 0:256], AF.Copy)
            nc.vector.tensor_copy(ev[:, 256:264].bitcast(F32), ph[:, 256:260])
            nc.gpsimd.memset(ev[:, 264:TROW], 0.0)
            if w < ATILES:
                dst = tsA[l_next][:].rearrange("(t p) r -> p t r", p=128)[:, w, :]
            else:
                dst = tsB[l_next][:].rearrange("(t p) r -> p t r", p=128)[:, w - ATILES, :]
            nc.sync.dma_start(dst, ev[:])

        def finalize_window(l, w, pw, slh_nx, alw_nx):
            nh = HEADS if l < 2 else 1
            # den > 0 always: the analytic self-loop term contributes exp(..)
            rden = smpool.tile([128, 4], F32, tag="rden")
            nc.vector.reciprocal(rden[:, 0:nh], pw[:, 256 : 256 + nh])
            y = ypool.tile([128, 256], F32, tag="y")
            if l < 2:
                y_v = y[:].rearrange("p (c h) -> p c h", h=4)
                p_v = pw[:, 0:256].rearrange("p (c h) -> p c h", h=4)
                rd_b = rden[:].unsqueeze(1).broadcast_to([128, 64, 4])
                nc.vector.tensor_tensor(y_v, p_v, rd_b, mm.mult)
            else:
                rd_b = rden[:, 0:1].broadcast_to([128, 256])
                nc.vector.tensor_tensor(y[:], pw[:, 0:256], rd_b, mm.mult)
            bias_b = consts_sb[:, C_BIAS[l] : C_BIAS[l] + 256]
            # bias add fused with the LN mean accumulation
            s1 = smpool.tile([128, 1], F32, tag="s1")
            nc.vector.scalar_tensor_tensor(
                y[:], y[:], 0.0, bias_b, mm.add, mm.add, accum_out=s1[:]
            )
            ysq = ypool.tile([128, 256], F32, tag="ysq")
            ss = smpool.tile([128, 1], F32, tag="ss")
            if GAT_NOACC:
                nc.vector.tensor_tensor(ysq[:], y[:], y[:], mm.mult)
                nc.vector.tensor_reduce(ss[:], ysq[:], mybir.AxisListType.X, mm.add)
            else:
                nc.scalar.activation(ysq[:], y[:], AF.Square, accum_out=ss[:])
            mu = smpool.tile([128, 1], F32, tag="mu")
            nc.vector.tensor_scalar_mul(mu[:], s1[:], 1.0 / 256.0)
            mu2 = smpool.tile([128, 1], F32, tag="mu2")
            nc.vector.tensor_tensor(mu2[:], mu[:], mu[:], mm.mult)
            var = smpool.tile([128, 1], F32, tag="var")
            nc.vector.scalar_tensor_tensor(
                var[:], ss[:], 1.0 / 256.0, mu2[:], mm.mult, mm.subtract
            )
            sd = smpool.tile([128, 1], F32, tag="sd")
            nc.scalar.activation(
                sd[:], var[:], AF.Sqrt, bias=consts_sb[:, C_EPS : C_EPS + 1]
            )
            rstd = smpool.tile([128, 1], F32, tag="rstd")
            nc.vector.reciprocal(rstd[:], sd[:])
            nmr = smpool.tile([128, 1], F32, tag="nmr")
            nc.vector.scalar_tensor_tensor(
                nmr[:], mu[:], -1.0, rstd[:], mm.mult, mm.mult
            )
            # y1 = y*rstd - mu*rstd
            nc.scalar.activation(
                y[:], y[:], AF.Identity, bias=nmr[:], scale=rstd[:]
            )
            nc.vector.tensor_tensor(
                y[:], y[:], consts_sb[:, C_G[l] : C_G[l] + 256], mm.mult
            )
            nc.vector.tensor_tensor(
                y[:], y[:], consts_sb[:, C_BE[l] : C_BE[l] + 256], mm.add
            )
            # ELU: max(y,0) + min(exp(y),1) - 1
            ee = ypool.tile([128, 256], F32, tag="ee")
            nc.scalar.activation(ee[:], y[:], AF.Exp)
            nc.vector.tensor_scalar_min(ee[:], ee[:], 1.0)
            nc.vector.tensor_scalar_max(y[:], y[:], 0.0)
            nc.vector.scalar_tensor_tensor(y[:], y[:], -1.0, ee[:], mm.add, mm.add)

            if l + 1 < GAT_LAYERS:
                xtw = xpool.tile([128, 2, 128], BF16, tag="xtw")
                for kc in range(2):
                    pt = pdt.tile([128, 128], F32, tag="pt")
                    nc.tensor.transpose(pt[:], y[:, kc * 128 : (kc + 1) * 128], ident_sb[:])
                    nc.scalar.activation(xtw[:, kc, :], pt[:], AF.Copy)
                emit_h(l + 1, w, xtw, slh_nx, alw_nx)
                if w == TRIG_A_W and not GAT_AGLATE:
                    ag(l + 1, 0)
            if l == GAT_LAYERS - 1:
                cmbf = smpool.tile([128, NG], BF16, tag="cmbf")
                gid_b = gid_sb[:, w : w + 1].broadcast_to([128, NG])
                nc.vector.tensor_tensor(cmbf[:], gid_b, iota_sb[:, 0:NG], mm.is_equal)
                ybf = ypool.tile([128, 256], BF16, tag="ybf")
                nc.scalar.activation(ybf[:], y[:], AF.Copy)
                pp = pooled_ps["pp"]
                nc.tensor.matmul(
                    pp[:],
                    cmbf[:],
                    ybf[:],
                    start=(w == 0),
                    stop=(w == NT - 1),
                    skip_group_check=True,
                )

        # ---- prologue: table(0) from input x ----
        slh_cur = slh_tile()
        alw_cur = alw_tile()
        for w in range(NT):
            xtw = xpool.tile([128, 2, 128], BF16, tag="xtw")
            nc.sync.dma_start(xtw[:], xT0[:, :, w * 128 : (w + 1) * 128])
            emit_h(0, w, xtw, slh_cur, alw_cur)
            if w == ATILES - 1 and not GAT_AGLATE:
                ag(0, 0)
        if GAT_AGLATE:
            ag(0, 0)
        ag(0, 1)

        # ---- layer loops ----
        for l in range(GAT_LAYERS):
            nh = HEADS if l < 2 else 1
            if l + 1 < GAT_LAYERS:
                slh_nx = slh_tile()
                alw_nx = alw_tile()
            else:
                slh_nx = alw_nx = None
            if l == GAT_LAYERS - 1:
                pooled_ps["pp"] = prag.tile(
                    [NG, 256], F32, tag="pp", bufs=1, name="pp"
                )
            win_psum = {}
            gq = [0]
            for ci, (c0, cb) in enumerate(chunks):
                G = gpool.tile([128, CH, TROW], BF16, tag="G")
                # gather calls per piece-run, round-robined over SWDGE queues
                r0 = 0
                while r0 < cb:
                    pc = blocks[c0 + r0][1]
                    r1 = r0
                    while (
                        r1 < cb
                        and blocks[c0 + r1][1] == pc
                        and r1 - r0 < GCALL
                    ):
                        r1 += 1
                    nrun = (r1 - r0) * 128
                    nc.gpsimd.dma_gather(
                        G[:, r0:r1, :],
                        (tabA[l] if pc == 0 else tabB[l])[:],
                        sidx_sb[:, (c0 + r0) * 8 : (c0 + r1) * 8],
                        nrun,
                        nrun,
                        TROW,
                        queue_num=gq[0] % NQUEUES,
                    )
                    gq[0] += 1
                    r0 = r1
                # cmp masks (batched)
                cmp = cmpool.tile([128, CH, DW], BF16, tag="cmp")
                dl_b = (
                    dloc_bf[:, c0 : c0 + cb].unsqueeze(2).broadcast_to([128, cb, DW])
                )
                io_b = iota_bf[:].unsqueeze(1).broadcast_to([128, cb, DW])
                nc.vector.tensor_tensor(cmp[:, 0:cb, :], io_b, dl_b, mm.is_equal)
                # ALD expansion: cmpT via PE transpose of cmp, then matmul vs alw
                ald_ps = pald.tile([128, 4 * CH], F32, tag="ald")
                for s0 in range(0, cb, SUB):
                    sbk = min(SUB, cb - s0)
                    ct_ps = pct.tile([128, SUB, 128], BF16, tag="ct")
                    for j in range(sbk):
                        nc.tensor.matmul(
                            ct_ps[:, j, :],
                            cmp[:, s0 + j, :],
                            ident_bf[:],
                            is_transpose=True,
                            start=(j == 0),
                            stop=(j == sbk - 1),
                            skip_group_check=True,
                        )
                    cmpt = npool.tile([128, SUB, 128], BF16, tag="cmpt")
                    nc.vector.tensor_copy(cmpt[:, 0:sbk, :], ct_ps[:, 0:sbk, :])
                    for j in range(sbk):
                        b = s0 + j
                        nc.tensor.matmul(
                            ald_ps[:, nh * b : nh * b + nh],
                            cmpt[:, j, :],
                            alw_cur[:, blocks[c0 + b][0], 0:nh],
                            start=(b == 0),
                            stop=(b == cb - 1),
                            skip_group_check=True,
                        )
                # attention pointwise (batched per chunk)
                nar = npool.tile([128, CH * 4], F32, tag="nar")
                nar_v = nar[:, 0 : nh * cb].rearrange("p (b a) -> p b a", a=nh)
                als_v = G[:, 0:cb, 256:264].bitcast(F32)[:, :, 0:nh]
                ald_v = ald_ps[:, 0 : nh * cb].rearrange("p (b a) -> p b a", a=nh)
                nc.vector.tensor_tensor(nar_v, als_v, ald_v, mm.add)
                lrt = npool.tile([128, CH * 4], F32, tag="lrt")
                nc.vector.scalar_tensor_tensor(
                    lrt[:, 0 : nh * cb], nar[:, 0 : nh * cb], NEG,
                    nar[:, 0 : nh * cb], mm.mult, mm.max,
                )
                lrt_v = lrt[:, 0 : nh * cb].rearrange("p (b a) -> p b a", a=nh)
                nc.scalar.activation(
                    G[:, 0:cb, 256 : 256 + nh], lrt_v, AF.Exp
                )
                if nh == 1:
                    # replicate ex to cols 257:260 so prescale uses the fast
                    # 4-wide broadcast pattern
                    nc.vector.tensor_copy(
                        G[:, 0:cb, 257:260],
                        G[:, 0:cb, 256:257].broadcast_to([128, cb, 3]),
                    )
                # prescale h by ex
                g_v = G[:, 0:cb, 0:256].rearrange("p b (c h) -> p b c h", h=4)
                ex_b = (
                    G[:, 0:cb, 256:260].unsqueeze(2).broadcast_to([128, cb, 64, 4])
                )
                nc.vector.tensor_tensor(g_v, g_v, ex_b, mm.mult)
                # aggregation
                for b in range(cb):
                    wi, pc, first, last = blocks[c0 + b]
                    if first:
                        pw = prag.tile([128, 260], F32, tag="agg")
                        win_psum[wi] = pw
                        nc.tensor.matmul(
                            pw[:, 0 : 256 + nh],
                            ident_bf[:],
                            slh_cur[:, wi, 0 : 256 + nh],
                            start=True,
                            stop=False,
                            skip_group_check=True,
                        )
                    pw = win_psum[wi]
                    nc.tensor.matmul(
                        pw[:, 0 : 256 + nh],
                        cmp[:, b, :],
                        G[:, b, 0 : 256 + nh],
                        start=False,
                        stop=last,
                        skip_group_check=True,
                    )
                    if last:
                        finalize_window(l, wi, pw, slh_nx, alw_nx)
                        del win_psum[wi]
            if l + 1 < GAT_LAYERS:
                if GAT_AGLATE:
                    ag(l + 1, 0)
                ag(l + 1, 1)
            slh_cur, alw_cur = slh_nx, alw_nx

        # ---- tail: AllReduce pooled sums + FC ----
        pooled = smpool.tile([NG, 256], F32, tag="pooled")
        nc.vector.tensor_copy(pooled[:], pooled_ps["pp"][:])
        nc.sync.dma_start(ar_in[:], pooled[:])
        nc.gpsimd.collective_compute(
            "AllReduce",
            mm.add,
            replica_groups=[list(range(NCORES))],
            ins=[ar_in.opt()],
            outs=[ar_out.opt()],
        )
        pooled2 = smpool.tile([NG, 256], F32, tag="pooled2")
        nc.sync.dma_start(pooled2[:], ar_out[:])
        nc.vector.tensor_scalar_mul(
            pooled2[:], pooled2[:], consts_sb[0:NG, C_INV : C_INV + 1]
        )
        fcw_sb = wpool.tile([128, 2, 256], F32, tag="fcw")
        nc.sync.dma_start(fcw_sb[:], fcw.rearrange("(k p) j -> p k j", p=128))
        poolT = smpool.tile([128, 2, NG], F32, tag="poolT")
        for kc in range(2):
            pt2 = pdt.tile([128, 128], F32, tag="pt")
            nc.tensor.transpose(
                pt2[0:128, 0:NG],
                pooled2[:, kc * 128 : (kc + 1) * 128],
                ident_sb[0:NG, 0:NG],
            )
            nc.vector.tensor_copy(poolT[:, kc, :], pt2[0:128, 0:NG])
        pfc = prag.tile([NG, 256], F32, tag="pp", bufs=1)
        for kc in range(2):
            nc.tensor.matmul(
                pfc[:],
                poolT[:, kc, :],
                fcw_sb[:, kc, :],
                start=(kc == 0),
                stop=(kc == 1),
            )
        ores = smpool.tile([NG, 256], F32, tag="ores")
        fcb_b = consts_sb[0:NG, C_FCB : C_FCB + 256]
        nc.vector.tensor_tensor(ores[:], pfc[:], fcb_b, mm.add)
        nc.vector.tensor_scalar_max(ores[:], ores[:], 0.0)
        nc.sync.dma_start(out_t[:], ores[:])

    nc.compile()
    return nc


def kernel(**inputs):
    x = np.asarray(inputs["x"], np.float32)
    edge_index = np.asarray(inputs["edge_index"])
    batch = np.asarray(inputs["batch"])

    blocks, NBLK, sidx, dloc, gid, inv_cnt = _prep_edges(edge_index, batch)
    wp = _prep_weights(inputs)

    key = (NBLK, CH, GAT_LAYERS, tuple(blocks[:8]))
    if key not in _PROGRAM_CACHE:
        _PROGRAM_CACHE[key] = _build_program(blocks, NBLK)
    nc = _PROGRAM_CACHE[key]

    iota = np.broadcast_to(np.arange(128, dtype=np.float32), (128, 128)).copy()
    ident = np.eye(128, dtype=np.float32)
    consts = np.zeros((128, CW), np.float32)
    for l in range(3):
        consts[:, C_BIAS[l] : C_BIAS[l] + 256] = wp[f"bias{l}"][None, :]
        consts[:, C_G[l] : C_G[l] + 256] = wp[f"g{l}"][None, :]
        consts[:, C_BE[l] : C_BE[l] + 256] = wp[f"be{l}"][None, :]
    consts[:, C_FCB : C_FCB + 256] = wp["fc_b"][None, :]
    consts[:NG, C_INV] = inv_cnt
    consts[NG:, C_INV] = 1.0
    consts[:, C_EPS] = EPS
    consts[:, C_R256] = 1.0 / 256.0
    consts[:, C_IOTAC] = np.arange(128, dtype=np.float32)
    for q in range(8):
        consts[q, C_SEL + 128 * q : C_SEL + 128 * (q + 1)] = 1.0

    in_maps = []
    for c in range(NCORES):
        xs = np.zeros((PADN, 256), np.float32)
        xs[:PARTN] = x[c * PARTN : (c + 1) * PARTN]
        xT0 = np.ascontiguousarray(
            xs.T.reshape(2, 128, PADN).transpose(1, 0, 2)
        ).astype(BF)
        in_maps.append(
            {
                "xT0": xT0,
                "wext0": wp["wext0"],
                "wext1": wp["wext1"],
                "wext2": wp["wext2"],
                "fcw": wp["fc_w"],
                "sidx": sidx[c],
                "dloc": dloc[c],
                "gid": gid[c],
                "iota": iota,
                "ident": ident,
                "consts": consts,
            }
        )

    global _LAST_RESULT
    res = run_bass_kernel_spmd(nc, in_maps, core_ids=list(range(NCORES)), trace=False)
    _LAST_RESULT = res
    return res.results[0]["out"]
